# revision 21
# baseline (speedup 1.0000x reference)
"""Trainium2 Bass kernel for LocalScopeSelfAttention (3x3 window, clamp-padded).

Shapes (hardcoded): x [2, 8, 32, 32, 256] f32, 8 heads x hd=32, LN eps 1e-5.
Sharding: data-parallel over B*T=16 frames -> 2 frames per core on 8 cores.

v6: engine-balance + startup restructure.
  - qst zero-stripes and vau ones come from DRAM constant inputs via DMA
    (no multi-microsecond DVE/GpSimd memsets on the critical path).
  - exp and the clamp-multiplicity mask run per subtile-PAIR ([128,1024]);
    masks on GpSimd (only contends with DVE 2-port ops, which are scarce).
  - xn transposes via the DMA xbar (off the tensor engine); attention output
    transposes stay on PE with ACT evacuation through the shared PSUM ring.
  - The two frames are pair-interleaved through attention; identity-transpose
    warm-up opens the PE clock gate during startup.
"""

import numpy as np
import ml_dtypes

H = W = 32
N = H * W          # 1024 tokens per frame
D = 256
NH, HD = 8, 32
LN_EPS = 1e-5
N_CORES = 8
FPC = 2            # frames per core
NPAD = N + 64      # padded tokens (32 guard each side)

_COMPILED = None


# ---------------------------------------------------------------- host helpers
def _build_masks_np():
    colcount = np.zeros((W, W), np.float32)
    for qc in range(W):
        for dc in (-1, 0, 1):
            colcount[qc, min(max(qc + dc, 0), W - 1)] += 1
    rowcounts = np.zeros((3, 2, 4), np.float32)
    for v, s in ((0, 0), (1, 7), (2, 15)):
        for rq in (0, 1):
            for dh in (-1, 0, 1):
                tgt = min(max(2 * s + rq + dh, 0), H - 1)
                rowcounts[v, rq, tgt - (2 * s - 1)] += 1
    masks = np.zeros((128, 3, 64), np.float32)
    for p in range(128):
        rp, kc = p // 32, p % 32
        for j in range(64):
            rq, qc = j // 32, j % 32
            for v in range(3):
                masks[p, v, j] = rowcounts[v, rq, rp] * colcount[qc, kc]
    return masks.astype(ml_dtypes.bfloat16)


def _fold_params(inp):
    f32 = np.float32
    g = inp["ln_g"].astype(f32)
    lb = inp["ln_b"].astype(f32)
    s = f32(1.0 / np.sqrt(HD))
    wq = (g[:, None] * inp["wq"].astype(f32)) * s
    bq = (lb @ inp["wq"].astype(f32) + inp["bq"].astype(f32)) * s
    wk = g[:, None] * inp["wk"].astype(f32)
    wv = g[:, None] * inp["wv"].astype(f32)
    bv = lb @ inp["wv"].astype(f32) + inp["bv"].astype(f32)
    wo = inp["wo"].astype(f32)
    bo = bv @ wo + inp["bo"].astype(f32)
    bf = ml_dtypes.bfloat16
    def wfmt(w):
        return np.ascontiguousarray(w.reshape(2, 128, 256).transpose(1, 0, 2)).astype(bf)
    return {
        "wq": wfmt(wq), "wk": wfmt(wk), "wv": wfmt(wv), "wo": wfmt(wo),
        "bq": np.ascontiguousarray(bq.reshape(2, 128).T).astype(f32),
        "bo": bo.reshape(1, 256).astype(bf),
        "masks": _build_masks_np(),
        "qz": np.zeros((128, FPC * 8192), ml_dtypes.bfloat16),
        "vz": np.ones((128, 9 * NH * 33), ml_dtypes.bfloat16),
    }


# ---------------------------------------------------------------- bass build
def _build_bass():
    from contextlib import ExitStack
    import concourse.tile as tile
    from concourse import bacc, mybir

    dt = mybir.dt
    AF = mybir.ActivationFunctionType
    OP = mybir.AluOpType

    nc = bacc.Bacc("TRN2", target_bir_lowering=False, debug=False,
                   num_devices=N_CORES)

    x_d = nc.dram_tensor("x", [FPC * N, D], dt.float32, kind="ExternalInput").ap()
    wq_d = nc.dram_tensor("wq", [128, 2, 256], dt.bfloat16, kind="ExternalInput").ap()
    wk_d = nc.dram_tensor("wk", [128, 2, 256], dt.bfloat16, kind="ExternalInput").ap()
    wv_d = nc.dram_tensor("wv", [128, 2, 256], dt.bfloat16, kind="ExternalInput").ap()
    wo_d = nc.dram_tensor("wo", [128, 2, 256], dt.bfloat16, kind="ExternalInput").ap()
    bq_d = nc.dram_tensor("bq", [128, 2], dt.float32, kind="ExternalInput").ap()
    bo_d = nc.dram_tensor("bo", [1, 256], dt.bfloat16, kind="ExternalInput").ap()
    mk_d = nc.dram_tensor("masks", [128, 3, 64], dt.bfloat16, kind="ExternalInput").ap()
    qz_d = nc.dram_tensor("qz", [128, FPC * 8192], dt.bfloat16, kind="ExternalInput").ap()
    vz_d = nc.dram_tensor("vz", [128, 9 * NH * 33], dt.bfloat16, kind="ExternalInput").ap()
    y_d = nc.dram_tensor("y", [FPC * N, D], dt.float32, kind="ExternalOutput").ap()

    with tile.TileContext(nc) as tc:
        with ExitStack() as ctx:
            const = ctx.enter_context(tc.tile_pool(name="const", bufs=1))
            frame = ctx.enter_context(tc.tile_pool(name="frame", bufs=1))
            work = ctx.enter_context(tc.tile_pool(name="work", bufs=3))
            att = ctx.enter_context(tc.tile_pool(name="att", bufs=3))
            psc = ctx.enter_context(tc.tile_pool(name="psc", bufs=2, space="PSUM"))
            pav = ctx.enter_context(tc.tile_pool(name="pav", bufs=2, space="PSUM"))
            pgen = ctx.enter_context(tc.tile_pool(name="pgen", bufs=2, space="PSUM"))

            # ---- constants ----
            ident = const.tile([128, 128], dt.bfloat16)
            from concourse.masks import make_identity
            make_identity(nc, ident[:])
            wq_s = const.tile([128, 2, 256], dt.bfloat16)
            wk_s = const.tile([128, 2, 256], dt.bfloat16)
            wv_s = const.tile([128, 2, 256], dt.bfloat16)
            wo_s = const.tile([128, 2, 256], dt.bfloat16)
            bq_s = const.tile([128, 2], dt.float32)
            bo_s = const.tile([1, 256], dt.bfloat16)
            mk_s = const.tile([128, 3, 64], dt.bfloat16)
            ones_s = const.tile([1, 1024], dt.bfloat16)
            nc.vector.memset(ones_s[:], 1.0)
            for cval in (0.0, LN_EPS):
                ct = const.tile([128, 1], dt.float32, tag=f"c{cval}")
                nc.vector.memset(ct[:], cval)
                nc.const_aps.aps[(dt.float32, cval)] = ct[:]

            # ---- persistent per-frame tensors (frame dim f explicit) ----
            x_f = frame.tile([128, FPC, 8, 256], dt.float32)
            xnT = frame.tile([128, FPC, 2, NPAD], dt.bfloat16)
            kTp = frame.tile([128, FPC, 2, NPAD], dt.bfloat16)
            qst = frame.tile([128, FPC, 2, 4, N], dt.bfloat16)
            vau = frame.tile([128, FPC, 9, NH, 33], dt.bfloat16)
            vau64 = frame.tile([128, FPC, 8, NH, 33], dt.bfloat16)
            xoT = frame.tile([128, FPC, 2, N], dt.bfloat16)
            ybuf = frame.tile([128, FPC, 8, 256], dt.float32)
            mv = frame.tile([128, FPC, 8, 2], dt.float32)
            rstd = frame.tile([128, FPC, 8], dt.float32)
            lnv = frame.tile([128, FPC, 8], dt.float32)

            # ---- input loads (per 2-chunk so LN pipelines behind the DMA) ----
            for f in range(FPC):
                for i in range(0, 8, 2):
                    nc.sync.dma_start(
                        x_f[:, f, i:i + 2, :],
                        x_d[f * N + 128 * i:f * N + 128 * (i + 2), :]
                        .rearrange("(i p) d -> p i d", p=128))
            # stripe-zero / ones init straight from DRAM (off the engines)
            for f in range(FPC):
                nc.sync.dma_start(
                    qst[:, f].rearrange("p a b c -> p (a b c)"),
                    qz_d[:, 8192 * f:8192 * (f + 1)])
                nc.sync.dma_start(
                    vau[:, f].rearrange("p a b c -> p (a b c)"), vz_d[:])
                nc.sync.dma_start(
                    vau64[:, f].rearrange("p a b c -> p (a b c)"),
                    vz_d[:, 0:8 * NH * 33])
            for sb, dd in ((wq_s, wq_d), (wk_s, wk_d), (wv_s, wv_d),
                           (wo_s, wo_d), (bq_s, bq_d), (bo_s, bo_d),
                           (mk_s, mk_d)):
                nc.sync.dma_start(sb[:], dd[:])

            # HAM warm-up: dense PE transposes during the otherwise-idle
            # startup window so the clock gate is open at the first matmul
            warm = pgen.tile([128, 256], dt.bfloat16, tag="gen")
            for _ in range(60):
                nc.tensor.transpose(warm[:, 0:128], ident[:], ident[:])

            # pad zeros for xnT / kTp (tiny)
            for f in range(FPC):
                for lo, hi in ((0, 32), (NPAD - 32, NPAD)):
                    nc.vector.memset(xnT[:, f, :, lo:hi], 0.0)
                    nc.vector.memset(kTp[:, f, :, lo:hi], 0.0)

            # ---------------- LN stats both frames ----------------
            for f in range(FPC):
                for i in range(8):
                    st = work.tile([128, 6], dt.float32, tag="bnst")
                    nc.vector.bn_stats(st[:], x_f[:, f, i, :])
                    nc.vector.bn_aggr(mv[:, f, i, :], st[:])
                nc.scalar.activation(lnv[:, f], mv[:, f, :, 1], AF.Ln,
                                     bias=LN_EPS, scale=1.0)
                nc.scalar.activation(rstd[:, f], lnv[:, f], AF.Exp,
                                     bias=0.0, scale=-0.5)

            # ------------- xn + DMA-xbar transpose into xnT -------------
            for f in range(FPC):
                for i in range(8):
                    xn = work.tile([128, 256], dt.bfloat16, tag="xn")
                    nc.vector.tensor_scalar(
                        xn[:], x_f[:, f, i, :], mv[:, f, i, 0:1],
                        rstd[:, f, i:i + 1], OP.subtract, OP.mult)
                    for kc in range(2):
                        nc.sync.dma_start(
                            xnT[:, f, kc, 32 + 128 * i:32 + 128 * (i + 1)],
                            xn[:, 128 * kc:128 * (kc + 1)], transpose=True)

            # ---------------- q & k & v projections ----------------
            for f in range(FPC):
                for mc in range(2):
                    qnat = work.tile([128, 1024], dt.bfloat16, tag="qnat")
                    for nh in range(2):
                        ns = slice(512 * nh, 512 * (nh + 1))
                        pq = pgen.tile([128, 512], dt.float32, tag="gen")
                        for kc in range(2):
                            nc.tensor.matmul(
                                pq[:], wq_s[:, kc, 128 * mc:128 * (mc + 1)],
                                xnT[:, f, kc, 32 + 512 * nh:32 + 512 * (nh + 1)],
                                start=(kc == 0), stop=(kc == 1))
                        nc.scalar.activation(qnat[:, ns], pq[:], AF.Identity,
                                             bias=bq_s[:, mc:mc + 1], scale=1.0)
                    for g in range(4):
                        nc.sync.dma_start(
                            qst[32 * g:32 * (g + 1), f, mc, g, :],
                            qnat[32 * g:32 * (g + 1), :])
                for mc in range(2):
                    for nh in range(2):
                        pk = pgen.tile([128, 512], dt.float32, tag="gen")
                        for kc in range(2):
                            nc.tensor.matmul(
                                pk[:], wk_s[:, kc, 128 * mc:128 * (mc + 1)],
                                xnT[:, f, kc, 32 + 512 * nh:32 + 512 * (nh + 1)],
                                start=(kc == 0), stop=(kc == 1))
                        nc.vector.tensor_copy(
                            kTp[:, f, mc, 32 + 512 * nh:32 + 512 * (nh + 1)],
                            pk[:])
                for c in range(9):
                    np_ = 128 if c < 8 else 64
                    pvv = pgen.tile([128, 256], dt.float32, tag="gen")
                    for kc in range(2):
                        nc.tensor.matmul(
                            pvv[0:np_, :], xnT[:, f, kc, 128 * c:128 * c + np_],
                            wv_s[:, kc, :], start=(kc == 0), stop=(kc == 1))
                    nc.scalar.copy(
                        vau[0:np_, f, c, :, 0:32],
                        pvv[0:np_, :].rearrange("p (h c) -> p h c", h=NH))
                nc.sync.dma_start(vau64[0:64, f], vau[64:128, f, 0:8])
                nc.sync.dma_start(vau64[64:128, f, 0:8], vau[0:64, f, 1:9])

            # -------- attention, frame-interleaved, out-proj inline --------
            ptro = [None, None]
            for p in range(8):
                for f in range(FPC):
                    pav_t = pav.tile([128, NH, 33], dt.float32, tag="pav")
                    pst = psc.tile([128, 2, 2, 4, 64], dt.float32, tag="sc")
                    for si in range(2):
                        s = 2 * p + si
                        for Q in range(2):
                            nc.tensor.matmul(
                                pst[:, si, Q, :, :],
                                kTp[:, f, Q, 64 * s:64 * s + 128],
                                qst[:, f, Q, :, 64 * s:64 * s + 64],
                                start=True, stop=True)
                    # paired exp + mask over [128, 1024]
                    ae = att.tile([128, NH, 2, 64], dt.bfloat16, tag="ae",
                                  bufs=3)
                    nc.scalar.activation(
                        ae[:].rearrange("p h s j -> p s h j"),
                        pst[:].rearrange("p s q g j -> p s (q g) j"),
                        AF.Exp, bias=0.0, scale=1.0)
                    am = att.tile([128, NH, 2, 64], dt.bfloat16, tag="am",
                                  bufs=4)
                    if p == 0:
                        mask_ap = (mk_s[:, 0:2, :].unsqueeze(1)
                                   .to_broadcast((128, NH, 2, 64)))
                    elif p == 7:
                        mask_ap = (mk_s[:, 1:3, :].unsqueeze(1)
                                   .to_broadcast((128, NH, 2, 64)))
                    else:
                        mask_ap = (mk_s[:, 1:2, :].unsqueeze(2)
                                   .to_broadcast((128, NH, 2, 64)))
                    nc.gpsimd.tensor_tensor(am[:], ae[:], mask_ap, OP.mult)
                    for si in range(2):
                        s = 2 * p + si
                        vsrc = (vau[:, f, s // 2] if si == 0
                                else vau64[:, f, (s - 1) // 2])
                        for h in range(NH):
                            nc.tensor.matmul(
                                pav_t[64 * si:64 * (si + 1), h, :],
                                am[:, h, si, :], vsrc[:, h, :],
                                start=True, stop=True)
                    rc = att.tile([128, NH], dt.float32, tag="rc")
                    nc.vector.reciprocal(rc[:], pav_t[:, :, 32])
                    onv = att.tile([128, NH, 32], dt.bfloat16, tag="onv")
                    nc.vector.tensor_tensor(
                        onv[:], pav_t[:, :, 0:32],
                        rc[:].unsqueeze(2).to_broadcast((128, NH, 32)),
                        OP.mult)
                    onf = onv[:].rearrange("p h c -> p (h c)")
                    if p % 2 == 0:
                        ptro[f] = pgen.tile([128, 4, 128], dt.bfloat16,
                                            tag="gen", name=f"ptro{f}")
                    for kc in range(2):
                        nc.tensor.transpose(
                            ptro[f][:, 2 * (p % 2) + kc, :],
                            onf[:, 128 * kc:128 * (kc + 1)], ident[:])
                    if p % 2 == 1:
                        u = p // 2
                        nc.scalar.copy(
                            xoT[:, f, :, 256 * u:256 * (u + 1)]
                            .rearrange("p mc (b c) -> p b mc c", b=2),
                            ptro[f][:].rearrange("p (b kc) c -> p b kc c", b=2))
                        for i in (2 * u, 2 * u + 1):
                            py = pgen.tile([128, 256], dt.float32, tag="gen")
                            for kc in range(2):
                                nc.tensor.matmul(
                                    py[:], xoT[:, f, kc, 128 * i:128 * (i + 1)],
                                    wo_s[:, kc, :], start=(kc == 0), stop=False)
                            nc.tensor.matmul(
                                py[:], ones_s[0:1, 0:128], bo_s[:],
                                start=False, stop=True)
                            nc.vector.tensor_tensor(
                                ybuf[:, f, i, :], py[:], x_f[:, f, i, :],
                                OP.add)
            # batched stores (2 x 4-chunk per frame)
            for f in range(FPC):
                for i in (0, 4):
                    nc.sync.dma_start(
                        y_d[f * N + 128 * i:f * N + 128 * (i + 4), :]
                        .rearrange("(i p) d -> p i d", p=128),
                        ybuf[:, f, i:i + 4, :])

    nc.compile()
    return nc


# ---------------------------------------------------------------- entry point
def kernel(**inputs):
    global _COMPILED
    if _COMPILED is None:
        _COMPILED = _build_bass()
    nc = _COMPILED

    from concourse.bass_utils import run_bass_kernel_spmd

    x = np.asarray(inputs["x"], dtype=np.float32)          # [2, 8, 32, 32, 256]
    B, T = x.shape[0], x.shape[1]
    frames = x.reshape(B * T, N, D)
    params = _fold_params({k: np.asarray(v) for k, v in inputs.items()})

    in_maps = []
    for c in range(N_CORES):
        m = {"x": np.ascontiguousarray(
            frames[FPC * c:FPC * (c + 1)].reshape(FPC * N, D))}
        m.update(params)
        in_maps.append(m)

    res = run_bass_kernel_spmd(nc, in_maps, list(range(N_CORES)))
    y = np.concatenate([res.results[c]["y"].reshape(FPC, N, D)
                        for c in range(N_CORES)], axis=0)
    return y.reshape(x.shape).astype(np.float32)


# revision 24
# speedup vs baseline: 1.3149x; 1.3149x over previous
"""Trainium2 Bass kernel for LocalScopeSelfAttention (3x3 window, clamp-padded).

Shapes (hardcoded): x [2, 8, 32, 32, 256] f32, 8 heads x hd=32, LN eps 1e-5.
Sharding: data-parallel over B*T=16 frames -> 2 frames per core on 8 cores.

v6: engine-balance + startup restructure.
  - qst zero-stripes and vau ones come from DRAM constant inputs via DMA
    (no multi-microsecond DVE/GpSimd memsets on the critical path).
  - exp and the clamp-multiplicity mask run per subtile-PAIR ([128,1024]);
    masks on GpSimd (only contends with DVE 2-port ops, which are scarce).
  - xn transposes via the DMA xbar (off the tensor engine); attention output
    transposes stay on PE with ACT evacuation through the shared PSUM ring.
  - The two frames are pair-interleaved through attention; identity-transpose
    warm-up opens the PE clock gate during startup.
"""

import numpy as np
import ml_dtypes

H = W = 32
N = H * W          # 1024 tokens per frame
D = 256
NH, HD = 8, 32
LN_EPS = 1e-5
N_CORES = 8
FPC = 2            # frames per core
NPAD = N + 64      # padded tokens (32 guard each side)

_COMPILED = None


# ---------------------------------------------------------------- host helpers
def _build_masks_np():
    colcount = np.zeros((W, W), np.float32)
    for qc in range(W):
        for dc in (-1, 0, 1):
            colcount[qc, min(max(qc + dc, 0), W - 1)] += 1
    rowcounts = np.zeros((3, 2, 4), np.float32)
    for v, s in ((0, 0), (1, 7), (2, 15)):
        for rq in (0, 1):
            for dh in (-1, 0, 1):
                tgt = min(max(2 * s + rq + dh, 0), H - 1)
                rowcounts[v, rq, tgt - (2 * s - 1)] += 1
    masks = np.zeros((128, 3, 64), np.float32)
    for p in range(128):
        rp, kc = p // 32, p % 32
        for j in range(64):
            rq, qc = j // 32, j % 32
            for v in range(3):
                masks[p, v, j] = rowcounts[v, rq, rp] * colcount[qc, kc]
    return masks.astype(ml_dtypes.bfloat16)


def _fold_params(inp):
    f32 = np.float32
    g = inp["ln_g"].astype(f32)
    lb = inp["ln_b"].astype(f32)
    s = f32(1.0 / np.sqrt(HD))
    wq = (g[:, None] * inp["wq"].astype(f32)) * s
    bq = (lb @ inp["wq"].astype(f32) + inp["bq"].astype(f32)) * s
    wk = g[:, None] * inp["wk"].astype(f32)
    wv = g[:, None] * inp["wv"].astype(f32)
    bv = lb @ inp["wv"].astype(f32) + inp["bv"].astype(f32)
    wo = inp["wo"].astype(f32)
    bo = bv @ wo + inp["bo"].astype(f32)
    bf = ml_dtypes.bfloat16
    def wfmt(w):
        return np.ascontiguousarray(w.reshape(2, 128, 256).transpose(1, 0, 2)).astype(bf)
    return {
        "wq": wfmt(wq), "wk": wfmt(wk), "wv": wfmt(wv), "wo": wfmt(wo),
        "bq": np.ascontiguousarray(bq.reshape(2, 128).T).astype(f32),
        "bo": bo.reshape(1, 256).astype(bf),
        "masks": _build_masks_np(),
        "qz": np.zeros((128, FPC * 8192), ml_dtypes.bfloat16),
        "vz": np.ones((128, 9 * NH * 33), ml_dtypes.bfloat16),
    }


# ---------------------------------------------------------------- bass build
def _build_bass():
    from contextlib import ExitStack
    import concourse.tile as tile
    from concourse import bacc, mybir

    dt = mybir.dt
    AF = mybir.ActivationFunctionType
    OP = mybir.AluOpType

    nc = bacc.Bacc("TRN2", target_bir_lowering=False, debug=False,
                   num_devices=N_CORES)

    x_d = nc.dram_tensor("x", [FPC * N, D], dt.float32, kind="ExternalInput").ap()
    wq_d = nc.dram_tensor("wq", [128, 2, 256], dt.bfloat16, kind="ExternalInput").ap()
    wk_d = nc.dram_tensor("wk", [128, 2, 256], dt.bfloat16, kind="ExternalInput").ap()
    wv_d = nc.dram_tensor("wv", [128, 2, 256], dt.bfloat16, kind="ExternalInput").ap()
    wo_d = nc.dram_tensor("wo", [128, 2, 256], dt.bfloat16, kind="ExternalInput").ap()
    bq_d = nc.dram_tensor("bq", [128, 2], dt.float32, kind="ExternalInput").ap()
    bo_d = nc.dram_tensor("bo", [1, 256], dt.bfloat16, kind="ExternalInput").ap()
    mk_d = nc.dram_tensor("masks", [128, 3, 64], dt.bfloat16, kind="ExternalInput").ap()
    qz_d = nc.dram_tensor("qz", [128, FPC * 8192], dt.bfloat16, kind="ExternalInput").ap()
    vz_d = nc.dram_tensor("vz", [128, 9 * NH * 33], dt.bfloat16, kind="ExternalInput").ap()
    y_d = nc.dram_tensor("y", [FPC * N, D], dt.float32, kind="ExternalOutput").ap()

    with tile.TileContext(nc) as tc:
        with ExitStack() as ctx:
            const = ctx.enter_context(tc.tile_pool(name="const", bufs=1))
            frame = ctx.enter_context(tc.tile_pool(name="frame", bufs=1))
            work = ctx.enter_context(tc.tile_pool(name="work", bufs=3))
            att = ctx.enter_context(tc.tile_pool(name="att", bufs=3))
            psc = ctx.enter_context(tc.tile_pool(name="psc", bufs=2, space="PSUM"))
            pav = ctx.enter_context(tc.tile_pool(name="pav", bufs=2, space="PSUM"))
            pgen = ctx.enter_context(tc.tile_pool(name="pgen", bufs=2, space="PSUM"))

            # ---- constants ----
            ident = const.tile([128, 128], dt.bfloat16)
            from concourse.masks import make_identity
            make_identity(nc, ident[:])
            wq_s = const.tile([128, 2, 256], dt.bfloat16)
            wk_s = const.tile([128, 2, 256], dt.bfloat16)
            wv_s = const.tile([128, 2, 256], dt.bfloat16)
            wo_s = const.tile([128, 2, 256], dt.bfloat16)
            bq_s = const.tile([128, 2], dt.float32)
            bo_s = const.tile([1, 256], dt.bfloat16)
            mk_s = const.tile([128, 3, 64], dt.bfloat16)
            ones_s = const.tile([1, 1024], dt.bfloat16)
            nc.vector.memset(ones_s[:], 1.0)
            for cval in (0.0, LN_EPS):
                ct = const.tile([128, 1], dt.float32, tag=f"c{cval}")
                nc.vector.memset(ct[:], cval)
                nc.const_aps.aps[(dt.float32, cval)] = ct[:]

            # ---- persistent per-frame tensors (frame dim f explicit) ----
            x_f = frame.tile([128, FPC, 8, 256], dt.float32)
            xnT = frame.tile([128, FPC, 2, NPAD], dt.bfloat16)
            kTp = frame.tile([128, FPC, 2, NPAD], dt.bfloat16)
            qst = frame.tile([128, FPC, 2, 4, N], dt.bfloat16)
            vau = frame.tile([128, FPC, 9, NH, 33], dt.bfloat16)
            vau64 = frame.tile([128, FPC, 8, NH, 33], dt.bfloat16)
            xoT = frame.tile([128, FPC, 2, N], dt.bfloat16)
            ybuf = frame.tile([128, FPC, 8, 256], dt.float32)
            mv = frame.tile([128, FPC, 8, 2], dt.float32)
            rstd = frame.tile([128, FPC, 8], dt.float32)
            lnv = frame.tile([128, FPC, 8], dt.float32)

            # ---- input loads (per 2-chunk so LN pipelines behind the DMA) ----
            for f in range(FPC):
                for i in range(0, 8, 2):
                    nc.sync.dma_start(
                        x_f[:, f, i:i + 2, :],
                        x_d[f * N + 128 * i:f * N + 128 * (i + 2), :]
                        .rearrange("(i p) d -> p i d", p=128))
            # stripe-zero / ones init straight from DRAM (off the engines)
            for f in range(FPC):
                nc.sync.dma_start(
                    qst[:, f].rearrange("p a b c -> p (a b c)"),
                    qz_d[:, 8192 * f:8192 * (f + 1)])
                nc.sync.dma_start(
                    vau[:, f].rearrange("p a b c -> p (a b c)"), vz_d[:])
                nc.sync.dma_start(
                    vau64[:, f].rearrange("p a b c -> p (a b c)"),
                    vz_d[:, 0:8 * NH * 33])
            for sb, dd in ((wq_s, wq_d), (wk_s, wk_d), (wv_s, wv_d),
                           (wo_s, wo_d), (bq_s, bq_d), (bo_s, bo_d),
                           (mk_s, mk_d)):
                nc.sync.dma_start(sb[:], dd[:])

            # HAM warm-up: dense PE transposes during the otherwise-idle
            # startup window so the clock gate is open at the first matmul
            warm = pgen.tile([128, 256], dt.bfloat16, tag="gen")
            for _ in range(60):
                nc.tensor.transpose(warm[:, 0:128], ident[:], ident[:])

            # pad zeros for xnT / kTp (tiny)
            for f in range(FPC):
                for lo, hi in ((0, 32), (NPAD - 32, NPAD)):
                    nc.vector.memset(xnT[:, f, :, lo:hi], 0.0)
                    nc.vector.memset(kTp[:, f, :, lo:hi], 0.0)

            # ---------------- LN stats both frames ----------------
            for f in range(FPC):
                for i in range(8):
                    st = work.tile([128, 6], dt.float32, tag="bnst")
                    nc.vector.bn_stats(st[:], x_f[:, f, i, :])
                    nc.vector.bn_aggr(mv[:, f, i, :], st[:])
                nc.scalar.activation(lnv[:, f], mv[:, f, :, 1], AF.Ln,
                                     bias=LN_EPS, scale=1.0)
                nc.scalar.activation(rstd[:, f], lnv[:, f], AF.Exp,
                                     bias=0.0, scale=-0.5)

            # ------------- xn + PE transpose into xnT -------------
            for f in range(FPC):
                for u in range(4):
                    ptr = pgen.tile([128, 4, 128], dt.bfloat16, tag="gen")
                    for i2 in range(2):
                        i = 2 * u + i2
                        xn = work.tile([128, 256], dt.bfloat16, tag="xn")
                        nc.vector.tensor_scalar(
                            xn[:], x_f[:, f, i, :], mv[:, f, i, 0:1],
                            rstd[:, f, i:i + 1], OP.subtract, OP.mult)
                        for kc in range(2):
                            nc.tensor.transpose(
                                ptr[:, 2 * i2 + kc, :],
                                xn[:, 128 * kc:128 * (kc + 1)], ident[:])
                    nc.scalar.copy(
                        xnT[:, f, :, 32 + 256 * u:32 + 256 * (u + 1)]
                        .rearrange("p mc (i2 c) -> p i2 mc c", i2=2),
                        ptr[:].rearrange("p (i2 kc) c -> p i2 kc c", i2=2))

            # ---------------- q & k & v projections ----------------
            for f in range(FPC):
                for mc in range(2):
                    qnat = work.tile([128, 1024], dt.bfloat16, tag="qnat")
                    for nh in range(2):
                        ns = slice(512 * nh, 512 * (nh + 1))
                        pq = pgen.tile([128, 512], dt.float32, tag="gen")
                        for kc in range(2):
                            nc.tensor.matmul(
                                pq[:], wq_s[:, kc, 128 * mc:128 * (mc + 1)],
                                xnT[:, f, kc, 32 + 512 * nh:32 + 512 * (nh + 1)],
                                start=(kc == 0), stop=(kc == 1))
                        nc.scalar.activation(qnat[:, ns], pq[:], AF.Identity,
                                             bias=bq_s[:, mc:mc + 1], scale=1.0)
                    for g in range(4):
                        nc.sync.dma_start(
                            qst[32 * g:32 * (g + 1), f, mc, g, :],
                            qnat[32 * g:32 * (g + 1), :])
                for mc in range(2):
                    for nh in range(2):
                        pk = pgen.tile([128, 512], dt.float32, tag="gen")
                        for kc in range(2):
                            nc.tensor.matmul(
                                pk[:], wk_s[:, kc, 128 * mc:128 * (mc + 1)],
                                xnT[:, f, kc, 32 + 512 * nh:32 + 512 * (nh + 1)],
                                start=(kc == 0), stop=(kc == 1))
                        nc.vector.tensor_copy(
                            kTp[:, f, mc, 32 + 512 * nh:32 + 512 * (nh + 1)],
                            pk[:])
                for c in range(9):
                    np_ = 128 if c < 8 else 64
                    pvv = pgen.tile([128, 256], dt.float32, tag="gen")
                    for kc in range(2):
                        nc.tensor.matmul(
                            pvv[0:np_, :], xnT[:, f, kc, 128 * c:128 * c + np_],
                            wv_s[:, kc, :], start=(kc == 0), stop=(kc == 1))
                    nc.scalar.copy(
                        vau[0:np_, f, c, :, 0:32],
                        pvv[0:np_, :].rearrange("p (h c) -> p h c", h=NH))
                nc.sync.dma_start(vau64[0:64, f], vau[64:128, f, 0:8])
                nc.sync.dma_start(vau64[64:128, f, 0:8], vau[0:64, f, 1:9])

            # -------- attention, frame-interleaved, out-proj inline --------
            ptro = [None, None]
            for p in range(8):
                for f in range(FPC):
                    pav_t = pav.tile([128, NH, 33], dt.float32, tag="pav")
                    pst = psc.tile([128, 2, 2, 4, 64], dt.float32, tag="sc")
                    for si in range(2):
                        s = 2 * p + si
                        for Q in range(2):
                            nc.tensor.matmul(
                                pst[:, si, Q, :, :],
                                kTp[:, f, Q, 64 * s:64 * s + 128],
                                qst[:, f, Q, :, 64 * s:64 * s + 64],
                                start=True, stop=True)
                    # paired exp + mask over [128, 1024]
                    ae = att.tile([128, NH, 2, 64], dt.bfloat16, tag="ae",
                                  bufs=3)
                    nc.scalar.activation(
                        ae[:].rearrange("p h s j -> p s h j"),
                        pst[:].rearrange("p s q g j -> p s (q g) j"),
                        AF.Exp, bias=0.0, scale=1.0)
                    am = att.tile([128, NH, 2, 64], dt.bfloat16, tag="am",
                                  bufs=4)
                    if p == 0:
                        mask_ap = (mk_s[:, 0:2, :].unsqueeze(1)
                                   .to_broadcast((128, NH, 2, 64)))
                    elif p == 7:
                        mask_ap = (mk_s[:, 1:3, :].unsqueeze(1)
                                   .to_broadcast((128, NH, 2, 64)))
                    else:
                        mask_ap = (mk_s[:, 1:2, :].unsqueeze(2)
                                   .to_broadcast((128, NH, 2, 64)))
                    nc.gpsimd.tensor_tensor(am[:], ae[:], mask_ap, OP.mult)
                    for si in range(2):
                        s = 2 * p + si
                        vsrc = (vau[:, f, s // 2] if si == 0
                                else vau64[:, f, (s - 1) // 2])
                        for h in range(NH):
                            nc.tensor.matmul(
                                pav_t[64 * si:64 * (si + 1), h, :],
                                am[:, h, si, :], vsrc[:, h, :],
                                start=True, stop=True)
                    rc = att.tile([128, NH], dt.float32, tag="rc")
                    nc.vector.reciprocal(rc[:], pav_t[:, :, 32])
                    onv = att.tile([128, NH, 32], dt.bfloat16, tag="onv")
                    nc.vector.tensor_tensor(
                        onv[:], pav_t[:, :, 0:32],
                        rc[:].unsqueeze(2).to_broadcast((128, NH, 32)),
                        OP.mult)
                    onf = onv[:].rearrange("p h c -> p (h c)")
                    if p % 2 == 0:
                        ptro[f] = pgen.tile([128, 4, 128], dt.bfloat16,
                                            tag="gen", name=f"ptro{f}")
                    for kc in range(2):
                        nc.tensor.transpose(
                            ptro[f][:, 2 * (p % 2) + kc, :],
                            onf[:, 128 * kc:128 * (kc + 1)], ident[:])
                    if p % 2 == 1:
                        u = p // 2
                        nc.scalar.copy(
                            xoT[:, f, :, 256 * u:256 * (u + 1)]
                            .rearrange("p mc (b c) -> p b mc c", b=2),
                            ptro[f][:].rearrange("p (b kc) c -> p b kc c", b=2))
                        for i in (2 * u, 2 * u + 1):
                            py = pgen.tile([128, 256], dt.float32, tag="gen")
                            for kc in range(2):
                                nc.tensor.matmul(
                                    py[:], xoT[:, f, kc, 128 * i:128 * (i + 1)],
                                    wo_s[:, kc, :], start=(kc == 0), stop=False)
                            nc.tensor.matmul(
                                py[:], ones_s[0:1, 0:128], bo_s[:],
                                start=False, stop=True)
                            nc.vector.tensor_tensor(
                                ybuf[:, f, i, :], py[:], x_f[:, f, i, :],
                                OP.add)
            # batched stores (2 x 4-chunk per frame)
            for f in range(FPC):
                for i in (0, 4):
                    nc.sync.dma_start(
                        y_d[f * N + 128 * i:f * N + 128 * (i + 4), :]
                        .rearrange("(i p) d -> p i d", p=128),
                        ybuf[:, f, i:i + 4, :])

    nc.compile()
    return nc


# ---------------------------------------------------------------- entry point
def kernel(**inputs):
    global _COMPILED
    if _COMPILED is None:
        _COMPILED = _build_bass()
    nc = _COMPILED

    from concourse.bass_utils import run_bass_kernel_spmd

    x = np.asarray(inputs["x"], dtype=np.float32)          # [2, 8, 32, 32, 256]
    B, T = x.shape[0], x.shape[1]
    frames = x.reshape(B * T, N, D)
    params = _fold_params({k: np.asarray(v) for k, v in inputs.items()})

    in_maps = []
    for c in range(N_CORES):
        m = {"x": np.ascontiguousarray(
            frames[FPC * c:FPC * (c + 1)].reshape(FPC * N, D))}
        m.update(params)
        in_maps.append(m)

    res = run_bass_kernel_spmd(nc, in_maps, list(range(N_CORES)))
    y = np.concatenate([res.results[c]["y"].reshape(FPC, N, D)
                        for c in range(N_CORES)], axis=0)
    return y.reshape(x.shape).astype(np.float32)


# revision 26
# speedup vs baseline: 1.4302x; 1.0877x over previous
"""Trainium2 Bass kernel for LocalScopeSelfAttention (3x3 window, clamp-padded).

Shapes (hardcoded): x [2, 8, 32, 32, 256] f32, 8 heads x hd=32, LN eps 1e-5.
Sharding: data-parallel over B*T=16 frames -> 2 frames per core on 8 cores.

v6: engine-balance + startup restructure.
  - qst zero-stripes and vau ones come from DRAM constant inputs via DMA
    (no multi-microsecond DVE/GpSimd memsets on the critical path).
  - exp and the clamp-multiplicity mask run per subtile-PAIR ([128,1024]);
    masks on GpSimd (only contends with DVE 2-port ops, which are scarce).
  - xn transposes via the DMA xbar (off the tensor engine); attention output
    transposes stay on PE with ACT evacuation through the shared PSUM ring.
  - The two frames are pair-interleaved through attention; identity-transpose
    warm-up opens the PE clock gate during startup.
"""

import numpy as np
import ml_dtypes

H = W = 32
N = H * W          # 1024 tokens per frame
D = 256
NH, HD = 8, 32
LN_EPS = 1e-5
N_CORES = 8
FPC = 2            # frames per core
NPAD = N + 64      # padded tokens (32 guard each side)

_COMPILED = None


# ---------------------------------------------------------------- host helpers
def _build_masks_np():
    colcount = np.zeros((W, W), np.float32)
    for qc in range(W):
        for dc in (-1, 0, 1):
            colcount[qc, min(max(qc + dc, 0), W - 1)] += 1
    rowcounts = np.zeros((3, 2, 4), np.float32)
    for v, s in ((0, 0), (1, 7), (2, 15)):
        for rq in (0, 1):
            for dh in (-1, 0, 1):
                tgt = min(max(2 * s + rq + dh, 0), H - 1)
                rowcounts[v, rq, tgt - (2 * s - 1)] += 1
    masks = np.zeros((128, 3, 64), np.float32)
    for p in range(128):
        rp, kc = p // 32, p % 32
        for j in range(64):
            rq, qc = j // 32, j % 32
            for v in range(3):
                masks[p, v, j] = rowcounts[v, rq, rp] * colcount[qc, kc]
    return masks.astype(ml_dtypes.bfloat16)


def _fold_params(inp):
    f32 = np.float32
    g = inp["ln_g"].astype(f32)
    lb = inp["ln_b"].astype(f32)
    s = f32(1.0 / np.sqrt(HD))
    wq = (g[:, None] * inp["wq"].astype(f32)) * s
    bq = (lb @ inp["wq"].astype(f32) + inp["bq"].astype(f32)) * s
    wk = g[:, None] * inp["wk"].astype(f32)
    wv = g[:, None] * inp["wv"].astype(f32)
    bv = lb @ inp["wv"].astype(f32) + inp["bv"].astype(f32)
    wo = inp["wo"].astype(f32)
    bo = bv @ wo + inp["bo"].astype(f32)
    bf = ml_dtypes.bfloat16
    def wfmt(w):
        return np.ascontiguousarray(w.reshape(2, 128, 256).transpose(1, 0, 2)).astype(bf)
    return {
        "wq": wfmt(wq), "wk": wfmt(wk), "wv": wfmt(wv), "wo": wfmt(wo),
        "bq": np.ascontiguousarray(bq.reshape(2, 128).T).astype(f32),
        "bo": bo.reshape(1, 256).astype(bf),
        "masks": _build_masks_np(),
        "qz": np.zeros((128, FPC * 8192), ml_dtypes.bfloat16),
        "vz": np.ones((128, 9 * NH * 33), ml_dtypes.bfloat16),
    }


# ---------------------------------------------------------------- bass build
def _build_bass():
    from contextlib import ExitStack
    import concourse.tile as tile
    from concourse import bacc, mybir

    dt = mybir.dt
    AF = mybir.ActivationFunctionType
    OP = mybir.AluOpType

    nc = bacc.Bacc("TRN2", target_bir_lowering=False, debug=False,
                   num_devices=N_CORES)

    x_d = nc.dram_tensor("x", [FPC * N, D], dt.float32, kind="ExternalInput").ap()
    wq_d = nc.dram_tensor("wq", [128, 2, 256], dt.bfloat16, kind="ExternalInput").ap()
    wk_d = nc.dram_tensor("wk", [128, 2, 256], dt.bfloat16, kind="ExternalInput").ap()
    wv_d = nc.dram_tensor("wv", [128, 2, 256], dt.bfloat16, kind="ExternalInput").ap()
    wo_d = nc.dram_tensor("wo", [128, 2, 256], dt.bfloat16, kind="ExternalInput").ap()
    bq_d = nc.dram_tensor("bq", [128, 2], dt.float32, kind="ExternalInput").ap()
    bo_d = nc.dram_tensor("bo", [1, 256], dt.bfloat16, kind="ExternalInput").ap()
    mk_d = nc.dram_tensor("masks", [128, 3, 64], dt.bfloat16, kind="ExternalInput").ap()
    qz_d = nc.dram_tensor("qz", [128, FPC * 8192], dt.bfloat16, kind="ExternalInput").ap()
    vz_d = nc.dram_tensor("vz", [128, 9 * NH * 33], dt.bfloat16, kind="ExternalInput").ap()
    y_d = nc.dram_tensor("y", [FPC * N, D], dt.float32, kind="ExternalOutput").ap()

    with tile.TileContext(nc) as tc:
        with ExitStack() as ctx:
            const = ctx.enter_context(tc.tile_pool(name="const", bufs=1))
            frame = ctx.enter_context(tc.tile_pool(name="frame", bufs=1))
            work = ctx.enter_context(tc.tile_pool(name="work", bufs=3))
            att = ctx.enter_context(tc.tile_pool(name="att", bufs=3))
            psc = ctx.enter_context(tc.tile_pool(name="psc", bufs=2, space="PSUM"))
            pav = ctx.enter_context(tc.tile_pool(name="pav", bufs=2, space="PSUM"))
            pgen = ctx.enter_context(tc.tile_pool(name="pgen", bufs=2, space="PSUM"))

            # ---- constants ----
            ident = const.tile([128, 128], dt.bfloat16)
            from concourse.masks import make_identity
            make_identity(nc, ident[:])
            wq_s = const.tile([128, 2, 256], dt.bfloat16)
            wk_s = const.tile([128, 2, 256], dt.bfloat16)
            wv_s = const.tile([128, 2, 256], dt.bfloat16)
            wo_s = const.tile([128, 2, 256], dt.bfloat16)
            bq_s = const.tile([128, 2], dt.float32)
            bo_s = const.tile([1, 256], dt.bfloat16)
            mk_s = const.tile([128, 3, 64], dt.bfloat16)
            ones_s = const.tile([1, 1024], dt.bfloat16)
            nc.vector.memset(ones_s[:], 1.0)
            for cval in (0.0, LN_EPS):
                ct = const.tile([128, 1], dt.float32, tag=f"c{cval}")
                nc.vector.memset(ct[:], cval)
                nc.const_aps.aps[(dt.float32, cval)] = ct[:]

            # ---- persistent per-frame tensors (frame dim f explicit) ----
            x_f = frame.tile([128, FPC, 8, 256], dt.float32)
            xnT = frame.tile([128, FPC, 2, NPAD], dt.bfloat16)
            kTp = frame.tile([128, FPC, 2, NPAD], dt.bfloat16)
            qst = frame.tile([128, FPC, 2, 4, N], dt.bfloat16)
            vau = frame.tile([128, FPC, 9, NH, 33], dt.bfloat16)
            vau64 = frame.tile([128, FPC, 8, NH, 33], dt.bfloat16)
            xoT = frame.tile([128, FPC, 2, N], dt.bfloat16)
            ybuf = frame.tile([128, FPC, 8, 256], dt.float32)
            mv = frame.tile([128, FPC, 8, 2], dt.float32)
            rstd = frame.tile([128, FPC, 8], dt.float32)
            lnv = frame.tile([128, FPC, 8], dt.float32)

            # ---- input loads (per 2-chunk so LN pipelines behind the DMA) ----
            for f in range(FPC):
                for i in range(0, 8, 2):
                    nc.sync.dma_start(
                        x_f[:, f, i:i + 2, :],
                        x_d[f * N + 128 * i:f * N + 128 * (i + 2), :]
                        .rearrange("(i p) d -> p i d", p=128))
            # stripe-zero / ones init straight from DRAM (off the engines)
            for f in range(FPC):
                nc.sync.dma_start(
                    qst[:, f].rearrange("p a b c -> p (a b c)"),
                    qz_d[:, 8192 * f:8192 * (f + 1)])
                nc.sync.dma_start(
                    vau[:, f].rearrange("p a b c -> p (a b c)"), vz_d[:])
                nc.sync.dma_start(
                    vau64[:, f].rearrange("p a b c -> p (a b c)"),
                    vz_d[:, 0:8 * NH * 33])
            for sb, dd in ((wq_s, wq_d), (wk_s, wk_d), (wv_s, wv_d),
                           (wo_s, wo_d), (bq_s, bq_d), (bo_s, bo_d),
                           (mk_s, mk_d)):
                nc.sync.dma_start(sb[:], dd[:])

            # HAM warm-up: dense PE transposes during the otherwise-idle
            # startup window so the clock gate is open at the first matmul
            warm = pgen.tile([128, 256], dt.bfloat16, tag="gen")
            for _ in range(80):
                nc.tensor.transpose(warm[:, 0:128], ident[:], ident[:])

            # pad zeros for xnT / kTp (tiny)
            for f in range(FPC):
                for lo, hi in ((0, 32), (NPAD - 32, NPAD)):
                    nc.vector.memset(xnT[:, f, :, lo:hi], 0.0)
                    nc.vector.memset(kTp[:, f, :, lo:hi], 0.0)

            # ------------- LN + xn + PE transpose into xnT, per frame ------
            for f in range(FPC):
                for i in range(8):
                    st = work.tile([128, 6], dt.float32, tag="bnst")
                    nc.vector.bn_stats(st[:], x_f[:, f, i, :])
                    nc.vector.bn_aggr(mv[:, f, i, :], st[:])
                nc.scalar.activation(lnv[:, f], mv[:, f, :, 1], AF.Ln,
                                     bias=LN_EPS, scale=1.0)
                nc.scalar.activation(rstd[:, f], lnv[:, f], AF.Exp,
                                     bias=0.0, scale=-0.5)
                for u in range(4):
                    ptr = pgen.tile([128, 4, 128], dt.bfloat16, tag="gen")
                    for i2 in range(2):
                        i = 2 * u + i2
                        xn = work.tile([128, 256], dt.bfloat16, tag="xn")
                        nc.vector.tensor_scalar(
                            xn[:], x_f[:, f, i, :], mv[:, f, i, 0:1],
                            rstd[:, f, i:i + 1], OP.subtract, OP.mult)
                        for kc in range(2):
                            nc.tensor.transpose(
                                ptr[:, 2 * i2 + kc, :],
                                xn[:, 128 * kc:128 * (kc + 1)], ident[:])
                    nc.scalar.copy(
                        xnT[:, f, :, 32 + 256 * u:32 + 256 * (u + 1)]
                        .rearrange("p mc (i2 c) -> p i2 mc c", i2=2),
                        ptr[:].rearrange("p (i2 kc) c -> p i2 kc c", i2=2))

            # ---------------- q & k & v projections ----------------
            for f in range(FPC):
                for mc in range(2):
                    qnat = work.tile([128, 1024], dt.bfloat16, tag="qnat")
                    for nh in range(2):
                        ns = slice(512 * nh, 512 * (nh + 1))
                        pq = pgen.tile([128, 512], dt.float32, tag="gen")
                        for kc in range(2):
                            nc.tensor.matmul(
                                pq[:], wq_s[:, kc, 128 * mc:128 * (mc + 1)],
                                xnT[:, f, kc, 32 + 512 * nh:32 + 512 * (nh + 1)],
                                start=(kc == 0), stop=(kc == 1))
                        nc.scalar.activation(qnat[:, ns], pq[:], AF.Identity,
                                             bias=bq_s[:, mc:mc + 1], scale=1.0)
                    for g in range(4):
                        nc.sync.dma_start(
                            qst[32 * g:32 * (g + 1), f, mc, g, :],
                            qnat[32 * g:32 * (g + 1), :])
                for mc in range(2):
                    for nh in range(2):
                        pk = pgen.tile([128, 512], dt.float32, tag="gen")
                        for kc in range(2):
                            nc.tensor.matmul(
                                pk[:], wk_s[:, kc, 128 * mc:128 * (mc + 1)],
                                xnT[:, f, kc, 32 + 512 * nh:32 + 512 * (nh + 1)],
                                start=(kc == 0), stop=(kc == 1))
                        nc.vector.tensor_copy(
                            kTp[:, f, mc, 32 + 512 * nh:32 + 512 * (nh + 1)],
                            pk[:])
                for c in range(9):
                    np_ = 128 if c < 8 else 64
                    pvv = pgen.tile([128, 256], dt.float32, tag="gen")
                    for kc in range(2):
                        nc.tensor.matmul(
                            pvv[0:np_, :], xnT[:, f, kc, 128 * c:128 * c + np_],
                            wv_s[:, kc, :], start=(kc == 0), stop=(kc == 1))
                    nc.scalar.copy(
                        vau[0:np_, f, c, :, 0:32],
                        pvv[0:np_, :].rearrange("p (h c) -> p h c", h=NH))
                nc.sync.dma_start(vau64[0:64, f], vau[64:128, f, 0:8])
                nc.sync.dma_start(vau64[64:128, f, 0:8], vau[0:64, f, 1:9])

            # -------- attention, frame-interleaved, out-proj inline --------
            ptro = [None, None]
            for p in range(8):
                for f in range(FPC):
                    pav_t = pav.tile([128, NH, 33], dt.float32, tag="pav")
                    pst = psc.tile([128, 2, 2, 4, 64], dt.float32, tag="sc")
                    for si in range(2):
                        s = 2 * p + si
                        for Q in range(2):
                            nc.tensor.matmul(
                                pst[:, si, Q, :, :],
                                kTp[:, f, Q, 64 * s:64 * s + 128],
                                qst[:, f, Q, :, 64 * s:64 * s + 64],
                                start=True, stop=True)
                    # paired exp + mask over [128, 1024]
                    ae = att.tile([128, NH, 2, 64], dt.bfloat16, tag="ae",
                                  bufs=3)
                    nc.scalar.activation(
                        ae[:].rearrange("p h s j -> p s h j"),
                        pst[:].rearrange("p s q g j -> p s (q g) j"),
                        AF.Exp, bias=0.0, scale=1.0)
                    am = att.tile([128, NH, 2, 64], dt.bfloat16, tag="am",
                                  bufs=4)
                    if p == 0:
                        mask_ap = (mk_s[:, 0:2, :].unsqueeze(1)
                                   .to_broadcast((128, NH, 2, 64)))
                    elif p == 7:
                        mask_ap = (mk_s[:, 1:3, :].unsqueeze(1)
                                   .to_broadcast((128, NH, 2, 64)))
                    else:
                        mask_ap = (mk_s[:, 1:2, :].unsqueeze(2)
                                   .to_broadcast((128, NH, 2, 64)))
                    nc.gpsimd.tensor_tensor(am[:], ae[:], mask_ap, OP.mult)
                    for si in range(2):
                        s = 2 * p + si
                        vsrc = (vau[:, f, s // 2] if si == 0
                                else vau64[:, f, (s - 1) // 2])
                        for h in range(NH):
                            nc.tensor.matmul(
                                pav_t[64 * si:64 * (si + 1), h, :],
                                am[:, h, si, :], vsrc[:, h, :],
                                start=True, stop=True)
                    rc = att.tile([128, NH], dt.float32, tag="rc")
                    nc.vector.reciprocal(rc[:], pav_t[:, :, 32])
                    onv = att.tile([128, NH, 32], dt.bfloat16, tag="onv")
                    nc.vector.tensor_tensor(
                        onv[:], pav_t[:, :, 0:32],
                        rc[:].unsqueeze(2).to_broadcast((128, NH, 32)),
                        OP.mult)
                    onf = onv[:].rearrange("p h c -> p (h c)")
                    if p % 2 == 0:
                        ptro[f] = pgen.tile([128, 4, 128], dt.bfloat16,
                                            tag="gen", name=f"ptro{f}")
                    for kc in range(2):
                        nc.tensor.transpose(
                            ptro[f][:, 2 * (p % 2) + kc, :],
                            onf[:, 128 * kc:128 * (kc + 1)], ident[:])
                    if p % 2 == 1:
                        u = p // 2
                        nc.scalar.copy(
                            xoT[:, f, :, 256 * u:256 * (u + 1)]
                            .rearrange("p mc (b c) -> p b mc c", b=2),
                            ptro[f][:].rearrange("p (b kc) c -> p b kc c", b=2))
                        for i in (2 * u, 2 * u + 1):
                            py = pgen.tile([128, 256], dt.float32, tag="gen")
                            for kc in range(2):
                                nc.tensor.matmul(
                                    py[:], xoT[:, f, kc, 128 * i:128 * (i + 1)],
                                    wo_s[:, kc, :], start=(kc == 0), stop=False)
                            nc.tensor.matmul(
                                py[:], ones_s[0:1, 0:128], bo_s[:],
                                start=False, stop=True)
                            nc.vector.tensor_tensor(
                                ybuf[:, f, i, :], py[:], x_f[:, f, i, :],
                                OP.add)
            # batched stores (2 x 4-chunk per frame)
            for f in range(FPC):
                for i in (0, 4):
                    nc.sync.dma_start(
                        y_d[f * N + 128 * i:f * N + 128 * (i + 4), :]
                        .rearrange("(i p) d -> p i d", p=128),
                        ybuf[:, f, i:i + 4, :])

    nc.compile()
    return nc


# ---------------------------------------------------------------- entry point
def kernel(**inputs):
    global _COMPILED
    if _COMPILED is None:
        _COMPILED = _build_bass()
    nc = _COMPILED

    from concourse.bass_utils import run_bass_kernel_spmd

    x = np.asarray(inputs["x"], dtype=np.float32)          # [2, 8, 32, 32, 256]
    B, T = x.shape[0], x.shape[1]
    frames = x.reshape(B * T, N, D)
    params = _fold_params({k: np.asarray(v) for k, v in inputs.items()})

    in_maps = []
    for c in range(N_CORES):
        m = {"x": np.ascontiguousarray(
            frames[FPC * c:FPC * (c + 1)].reshape(FPC * N, D))}
        m.update(params)
        in_maps.append(m)

    res = run_bass_kernel_spmd(nc, in_maps, list(range(N_CORES)))
    y = np.concatenate([res.results[c]["y"].reshape(FPC, N, D)
                        for c in range(N_CORES)], axis=0)
    return y.reshape(x.shape).astype(np.float32)


# revision 28
# speedup vs baseline: 1.4723x; 1.0294x over previous
"""Trainium2 Bass kernel for LocalScopeSelfAttention (3x3 window, clamp-padded).

Shapes (hardcoded): x [2, 8, 32, 32, 256] f32, 8 heads x hd=32, LN eps 1e-5.
Sharding: data-parallel over B*T=16 frames -> 2 frames per core on 8 cores.

v6: engine-balance + startup restructure.
  - qst zero-stripes and vau ones come from DRAM constant inputs via DMA
    (no multi-microsecond DVE/GpSimd memsets on the critical path).
  - exp and the clamp-multiplicity mask run per subtile-PAIR ([128,1024]);
    masks on GpSimd (only contends with DVE 2-port ops, which are scarce).
  - xn transposes via the DMA xbar (off the tensor engine); attention output
    transposes stay on PE with ACT evacuation through the shared PSUM ring.
  - The two frames are pair-interleaved through attention; identity-transpose
    warm-up opens the PE clock gate during startup.
"""

import numpy as np
import ml_dtypes

H = W = 32
N = H * W          # 1024 tokens per frame
D = 256
NH, HD = 8, 32
LN_EPS = 1e-5
N_CORES = 8
FPC = 2            # frames per core
NPAD = N + 64      # padded tokens (32 guard each side)

_COMPILED = None


# ---------------------------------------------------------------- host helpers
def _build_masks_np():
    colcount = np.zeros((W, W), np.float32)
    for qc in range(W):
        for dc in (-1, 0, 1):
            colcount[qc, min(max(qc + dc, 0), W - 1)] += 1
    rowcounts = np.zeros((3, 2, 4), np.float32)
    for v, s in ((0, 0), (1, 7), (2, 15)):
        for rq in (0, 1):
            for dh in (-1, 0, 1):
                tgt = min(max(2 * s + rq + dh, 0), H - 1)
                rowcounts[v, rq, tgt - (2 * s - 1)] += 1
    masks = np.zeros((128, 3, 64), np.float32)
    for p in range(128):
        rp, kc = p // 32, p % 32
        for j in range(64):
            rq, qc = j // 32, j % 32
            for v in range(3):
                masks[p, v, j] = rowcounts[v, rq, rp] * colcount[qc, kc]
    return masks.astype(ml_dtypes.bfloat16)


def _fold_params(inp):
    f32 = np.float32
    g = inp["ln_g"].astype(f32)
    lb = inp["ln_b"].astype(f32)
    s = f32(1.0 / np.sqrt(HD))
    wq = (g[:, None] * inp["wq"].astype(f32)) * s
    bq = (lb @ inp["wq"].astype(f32) + inp["bq"].astype(f32)) * s
    wk = g[:, None] * inp["wk"].astype(f32)
    wv = g[:, None] * inp["wv"].astype(f32)
    bv = lb @ inp["wv"].astype(f32) + inp["bv"].astype(f32)
    wo = inp["wo"].astype(f32)
    bo = bv @ wo + inp["bo"].astype(f32)
    bf = ml_dtypes.bfloat16
    def wfmt(w):
        return np.ascontiguousarray(w.reshape(2, 128, 256).transpose(1, 0, 2)).astype(bf)
    return {
        "wq": wfmt(wq), "wk": wfmt(wk), "wv": wfmt(wv), "wo": wfmt(wo),
        "bq": np.ascontiguousarray(bq.reshape(2, 128).T).astype(f32),
        "bo": bo.reshape(1, 256).astype(bf),
        "masks": _build_masks_np(),
        "qz": np.zeros((128, FPC * 8192), ml_dtypes.bfloat16),
        "vz": np.ones((128, 9 * NH * 33), ml_dtypes.bfloat16),
    }


# ---------------------------------------------------------------- bass build
def _build_bass():
    from contextlib import ExitStack
    import concourse.tile as tile
    from concourse import bacc, mybir

    dt = mybir.dt
    AF = mybir.ActivationFunctionType
    OP = mybir.AluOpType

    nc = bacc.Bacc("TRN2", target_bir_lowering=False, debug=False,
                   num_devices=N_CORES)

    x_d = nc.dram_tensor("x", [FPC * N, D], dt.float32, kind="ExternalInput").ap()
    wq_d = nc.dram_tensor("wq", [128, 2, 256], dt.bfloat16, kind="ExternalInput").ap()
    wk_d = nc.dram_tensor("wk", [128, 2, 256], dt.bfloat16, kind="ExternalInput").ap()
    wv_d = nc.dram_tensor("wv", [128, 2, 256], dt.bfloat16, kind="ExternalInput").ap()
    wo_d = nc.dram_tensor("wo", [128, 2, 256], dt.bfloat16, kind="ExternalInput").ap()
    bq_d = nc.dram_tensor("bq", [128, 2], dt.float32, kind="ExternalInput").ap()
    bo_d = nc.dram_tensor("bo", [1, 256], dt.bfloat16, kind="ExternalInput").ap()
    mk_d = nc.dram_tensor("masks", [128, 3, 64], dt.bfloat16, kind="ExternalInput").ap()
    qz_d = nc.dram_tensor("qz", [128, FPC * 8192], dt.bfloat16, kind="ExternalInput").ap()
    vz_d = nc.dram_tensor("vz", [128, 9 * NH * 33], dt.bfloat16, kind="ExternalInput").ap()
    y_d = nc.dram_tensor("y", [FPC * N, D], dt.float32, kind="ExternalOutput").ap()

    with tile.TileContext(nc) as tc:
        with ExitStack() as ctx:
            const = ctx.enter_context(tc.tile_pool(name="const", bufs=1))
            frame = ctx.enter_context(tc.tile_pool(name="frame", bufs=1))
            work = ctx.enter_context(tc.tile_pool(name="work", bufs=3))
            att = ctx.enter_context(tc.tile_pool(name="att", bufs=3))
            psc = ctx.enter_context(tc.tile_pool(name="psc", bufs=2, space="PSUM"))
            pav = ctx.enter_context(tc.tile_pool(name="pav", bufs=2, space="PSUM"))
            pgen = ctx.enter_context(tc.tile_pool(name="pgen", bufs=2, space="PSUM"))

            # ---- constants ----
            ident = const.tile([128, 128], dt.bfloat16)
            from concourse.masks import make_identity
            make_identity(nc, ident[:])
            wq_s = const.tile([128, 2, 256], dt.bfloat16)
            wk_s = const.tile([128, 2, 256], dt.bfloat16)
            wv_s = const.tile([128, 2, 256], dt.bfloat16)
            wo_s = const.tile([128, 2, 256], dt.bfloat16)
            bq_s = const.tile([128, 2], dt.float32)
            bo_s = const.tile([1, 256], dt.bfloat16)
            mk_s = const.tile([128, 3, 64], dt.bfloat16)
            ones_s = const.tile([1, 1024], dt.bfloat16)
            nc.vector.memset(ones_s[:], 1.0)
            for cval in (0.0, LN_EPS):
                ct = const.tile([128, 1], dt.float32, tag=f"c{cval}")
                nc.vector.memset(ct[:], cval)
                nc.const_aps.aps[(dt.float32, cval)] = ct[:]

            # ---- persistent per-frame tensors (frame dim f explicit) ----
            x_f = frame.tile([128, FPC, 8, 256], dt.float32)
            xnT = frame.tile([128, FPC, 2, NPAD], dt.bfloat16)
            kTp = frame.tile([128, FPC, 2, NPAD], dt.bfloat16)
            qst = frame.tile([128, FPC, 2, 4, N], dt.bfloat16)
            vau = frame.tile([128, FPC, 9, NH, 33], dt.bfloat16)
            vau64 = frame.tile([128, FPC, 8, NH, 33], dt.bfloat16)
            xoT = frame.tile([128, FPC, 2, N], dt.bfloat16)
            ybuf = frame.tile([128, FPC, 8, 256], dt.float32)
            mv = frame.tile([128, FPC, 8, 2], dt.float32)
            rstd = frame.tile([128, FPC, 8], dt.float32)
            lnv = frame.tile([128, FPC, 8], dt.float32)

            # ---- input loads (per 2-chunk so LN pipelines behind the DMA) ----
            for f in range(FPC):
                for i in range(0, 8, 2):
                    nc.sync.dma_start(
                        x_f[:, f, i:i + 2, :],
                        x_d[f * N + 128 * i:f * N + 128 * (i + 2), :]
                        .rearrange("(i p) d -> p i d", p=128))
            # stripe-zero / ones init straight from DRAM (off the engines)
            for f in range(FPC):
                nc.sync.dma_start(
                    qst[:, f].rearrange("p a b c -> p (a b c)"),
                    qz_d[:, 8192 * f:8192 * (f + 1)])
                nc.sync.dma_start(
                    vau[:, f].rearrange("p a b c -> p (a b c)"), vz_d[:])
                nc.sync.dma_start(
                    vau64[:, f].rearrange("p a b c -> p (a b c)"),
                    vz_d[:, 0:8 * NH * 33])
            for sb, dd in ((wq_s, wq_d), (wk_s, wk_d), (wv_s, wv_d),
                           (wo_s, wo_d), (bq_s, bq_d), (bo_s, bo_d),
                           (mk_s, mk_d)):
                nc.sync.dma_start(sb[:], dd[:])

            # HAM warm-up: dense PE transposes during the otherwise-idle
            # startup window so the clock gate is open at the first matmul
            warm = pgen.tile([128, 256], dt.bfloat16, tag="gen")
            for _ in range(80):
                nc.tensor.transpose(warm[:, 0:128], ident[:], ident[:])

            # pad zeros for xnT / kTp (tiny)
            for f in range(FPC):
                for lo, hi in ((0, 32), (NPAD - 32, NPAD)):
                    nc.vector.memset(xnT[:, f, :, lo:hi], 0.0)
                    nc.vector.memset(kTp[:, f, :, lo:hi], 0.0)

            # ------------- LN + xn + PE transpose into xnT, per frame ------
            for f in range(FPC):
                for i in range(8):
                    st = work.tile([128, 6], dt.float32, tag="bnst")
                    nc.vector.bn_stats(st[:], x_f[:, f, i, :])
                    nc.vector.bn_aggr(mv[:, f, i, :], st[:])
                nc.scalar.activation(lnv[:, f], mv[:, f, :, 1], AF.Ln,
                                     bias=LN_EPS, scale=1.0)
                nc.scalar.activation(rstd[:, f], lnv[:, f], AF.Exp,
                                     bias=0.0, scale=-0.5)
                for u in range(4):
                    ptr = pgen.tile([128, 4, 128], dt.bfloat16, tag="gen")
                    for i2 in range(2):
                        i = 2 * u + i2
                        xn = work.tile([128, 256], dt.bfloat16, tag="xn")
                        nc.vector.tensor_scalar(
                            xn[:], x_f[:, f, i, :], mv[:, f, i, 0:1],
                            rstd[:, f, i:i + 1], OP.subtract, OP.mult)
                        for kc in range(2):
                            nc.tensor.transpose(
                                ptr[:, 2 * i2 + kc, :],
                                xn[:, 128 * kc:128 * (kc + 1)], ident[:])
                    nc.scalar.copy(
                        xnT[:, f, :, 32 + 256 * u:32 + 256 * (u + 1)]
                        .rearrange("p mc (i2 c) -> p i2 mc c", i2=2),
                        ptr[:].rearrange("p (i2 kc) c -> p i2 kc c", i2=2))

            # ---------------- q & k & v projections ----------------
            for f in range(FPC):
                for mc in range(2):
                    qnat = work.tile([128, 1024], dt.bfloat16, tag="qnat")
                    for nh in range(2):
                        ns = slice(512 * nh, 512 * (nh + 1))
                        pq = pgen.tile([128, 512], dt.float32, tag="gen")
                        for kc in range(2):
                            nc.tensor.matmul(
                                pq[:], wq_s[:, kc, 128 * mc:128 * (mc + 1)],
                                xnT[:, f, kc, 32 + 512 * nh:32 + 512 * (nh + 1)],
                                start=(kc == 0), stop=(kc == 1))
                        nc.scalar.activation(qnat[:, ns], pq[:], AF.Identity,
                                             bias=bq_s[:, mc:mc + 1], scale=1.0)
                    for g in range(4):
                        nc.sync.dma_start(
                            qst[32 * g:32 * (g + 1), f, mc, g, :],
                            qnat[32 * g:32 * (g + 1), :])
                for mc in range(2):
                    for nh in range(2):
                        pk = pgen.tile([128, 512], dt.float32, tag="gen")
                        for kc in range(2):
                            nc.tensor.matmul(
                                pk[:], wk_s[:, kc, 128 * mc:128 * (mc + 1)],
                                xnT[:, f, kc, 32 + 512 * nh:32 + 512 * (nh + 1)],
                                start=(kc == 0), stop=(kc == 1))
                        nc.vector.tensor_copy(
                            kTp[:, f, mc, 32 + 512 * nh:32 + 512 * (nh + 1)],
                            pk[:])
                for c in range(9):
                    np_ = 128 if c < 8 else 64
                    pvv = pgen.tile([128, 256], dt.float32, tag="gen")
                    for kc in range(2):
                        nc.tensor.matmul(
                            pvv[0:np_, :], xnT[:, f, kc, 128 * c:128 * c + np_],
                            wv_s[:, kc, :], start=(kc == 0), stop=(kc == 1))
                    nc.scalar.copy(
                        vau[0:np_, f, c, :, 0:32],
                        pvv[0:np_, :].rearrange("p (h c) -> p h c", h=NH))
                nc.sync.dma_start(vau64[0:64, f], vau[64:128, f, 0:8])
                nc.sync.dma_start(vau64[64:128, f, 0:8], vau[0:64, f, 1:9])

            # -------- attention, frame-interleaved, out-proj inline --------
            ptro = [None, None]
            for p in range(8):
                for f in range(FPC):
                    pav_t = pav.tile([128, NH, 33], dt.float32, tag="pav")
                    pst = psc.tile([128, 2, 2, 4, 64], dt.float32, tag="sc")
                    for si in range(2):
                        s = 2 * p + si
                        for Q in range(2):
                            nc.tensor.matmul(
                                pst[:, si, Q, :, :],
                                kTp[:, f, Q, 64 * s:64 * s + 128],
                                qst[:, f, Q, :, 64 * s:64 * s + 64],
                                start=True, stop=True)
                    # paired exp + mask over [128, 1024]
                    ae = att.tile([128, NH, 2, 64], dt.bfloat16, tag="ae",
                                  bufs=3)
                    nc.scalar.activation(
                        ae[:].rearrange("p h s j -> p s h j"),
                        pst[:].rearrange("p s q g j -> p s (q g) j"),
                        AF.Exp, bias=0.0, scale=1.0)
                    am = att.tile([128, NH, 2, 64], dt.bfloat16, tag="am",
                                  bufs=6)
                    if p == 0:
                        mask_ap = (mk_s[:, 0:2, :].unsqueeze(1)
                                   .to_broadcast((128, NH, 2, 64)))
                    elif p == 7:
                        mask_ap = (mk_s[:, 1:3, :].unsqueeze(1)
                                   .to_broadcast((128, NH, 2, 64)))
                    else:
                        mask_ap = (mk_s[:, 1:2, :].unsqueeze(2)
                                   .to_broadcast((128, NH, 2, 64)))
                    meng = nc.vector if f == 0 else nc.gpsimd
                    meng.tensor_tensor(am[:], ae[:], mask_ap, OP.mult)
                    for si in range(2):
                        s = 2 * p + si
                        vsrc = (vau[:, f, s // 2] if si == 0
                                else vau64[:, f, (s - 1) // 2])
                        for h in range(NH):
                            nc.tensor.matmul(
                                pav_t[64 * si:64 * (si + 1), h, :],
                                am[:, h, si, :], vsrc[:, h, :],
                                start=True, stop=True)
                    rc = att.tile([128, NH], dt.float32, tag="rc")
                    nc.vector.reciprocal(rc[:], pav_t[:, :, 32])
                    onv = att.tile([128, NH, 32], dt.bfloat16, tag="onv")
                    nc.vector.tensor_tensor(
                        onv[:], pav_t[:, :, 0:32],
                        rc[:].unsqueeze(2).to_broadcast((128, NH, 32)),
                        OP.mult)
                    onf = onv[:].rearrange("p h c -> p (h c)")
                    if p % 2 == 0:
                        ptro[f] = pgen.tile([128, 4, 128], dt.bfloat16,
                                            tag="gen", name=f"ptro{f}")
                    for kc in range(2):
                        nc.tensor.transpose(
                            ptro[f][:, 2 * (p % 2) + kc, :],
                            onf[:, 128 * kc:128 * (kc + 1)], ident[:])
                    if p % 2 == 1:
                        u = p // 2
                        nc.scalar.copy(
                            xoT[:, f, :, 256 * u:256 * (u + 1)]
                            .rearrange("p mc (b c) -> p b mc c", b=2),
                            ptro[f][:].rearrange("p (b kc) c -> p b kc c", b=2))
                        for i in (2 * u, 2 * u + 1):
                            py = pgen.tile([128, 256], dt.float32, tag="gen")
                            for kc in range(2):
                                nc.tensor.matmul(
                                    py[:], xoT[:, f, kc, 128 * i:128 * (i + 1)],
                                    wo_s[:, kc, :], start=(kc == 0), stop=False)
                            nc.tensor.matmul(
                                py[:], ones_s[0:1, 0:128], bo_s[:],
                                start=False, stop=True)
                            nc.vector.tensor_tensor(
                                ybuf[:, f, i, :], py[:], x_f[:, f, i, :],
                                OP.add)
            # batched stores (2 x 4-chunk per frame)
            for f in range(FPC):
                for i in (0, 4):
                    nc.sync.dma_start(
                        y_d[f * N + 128 * i:f * N + 128 * (i + 4), :]
                        .rearrange("(i p) d -> p i d", p=128),
                        ybuf[:, f, i:i + 4, :])

    nc.compile()
    return nc


# ---------------------------------------------------------------- entry point
def kernel(**inputs):
    global _COMPILED
    if _COMPILED is None:
        _COMPILED = _build_bass()
    nc = _COMPILED

    from concourse.bass_utils import run_bass_kernel_spmd

    x = np.asarray(inputs["x"], dtype=np.float32)          # [2, 8, 32, 32, 256]
    B, T = x.shape[0], x.shape[1]
    frames = x.reshape(B * T, N, D)
    params = _fold_params({k: np.asarray(v) for k, v in inputs.items()})

    in_maps = []
    for c in range(N_CORES):
        m = {"x": np.ascontiguousarray(
            frames[FPC * c:FPC * (c + 1)].reshape(FPC * N, D))}
        m.update(params)
        in_maps.append(m)

    res = run_bass_kernel_spmd(nc, in_maps, list(range(N_CORES)))
    y = np.concatenate([res.results[c]["y"].reshape(FPC, N, D)
                        for c in range(N_CORES)], axis=0)
    return y.reshape(x.shape).astype(np.float32)


# revision 30
# speedup vs baseline: 1.5156x; 1.0294x over previous
"""Trainium2 Bass kernel for LocalScopeSelfAttention (3x3 window, clamp-padded).

Shapes (hardcoded): x [2, 8, 32, 32, 256] f32, 8 heads x hd=32, LN eps 1e-5.
Sharding: data-parallel over B*T=16 frames -> 2 frames per core on 8 cores.

v6: engine-balance + startup restructure.
  - qst zero-stripes and vau ones come from DRAM constant inputs via DMA
    (no multi-microsecond DVE/GpSimd memsets on the critical path).
  - exp and the clamp-multiplicity mask run per subtile-PAIR ([128,1024]);
    masks on GpSimd (only contends with DVE 2-port ops, which are scarce).
  - xn transposes via the DMA xbar (off the tensor engine); attention output
    transposes stay on PE with ACT evacuation through the shared PSUM ring.
  - The two frames are pair-interleaved through attention; identity-transpose
    warm-up opens the PE clock gate during startup.
"""

import numpy as np
import ml_dtypes

H = W = 32
N = H * W          # 1024 tokens per frame
D = 256
NH, HD = 8, 32
LN_EPS = 1e-5
N_CORES = 8
FPC = 2            # frames per core
NPAD = N + 64      # padded tokens (32 guard each side)

_COMPILED = None


# ---------------------------------------------------------------- host helpers
def _build_masks_np():
    colcount = np.zeros((W, W), np.float32)
    for qc in range(W):
        for dc in (-1, 0, 1):
            colcount[qc, min(max(qc + dc, 0), W - 1)] += 1
    rowcounts = np.zeros((3, 2, 4), np.float32)
    for v, s in ((0, 0), (1, 7), (2, 15)):
        for rq in (0, 1):
            for dh in (-1, 0, 1):
                tgt = min(max(2 * s + rq + dh, 0), H - 1)
                rowcounts[v, rq, tgt - (2 * s - 1)] += 1
    masks = np.zeros((128, 3, 64), np.float32)
    for p in range(128):
        rp, kc = p // 32, p % 32
        for j in range(64):
            rq, qc = j // 32, j % 32
            for v in range(3):
                masks[p, v, j] = rowcounts[v, rq, rp] * colcount[qc, kc]
    return masks.astype(ml_dtypes.bfloat16)


def _fold_params(inp):
    f32 = np.float32
    g = inp["ln_g"].astype(f32)
    lb = inp["ln_b"].astype(f32)
    s = f32(1.0 / np.sqrt(HD))
    wq = (g[:, None] * inp["wq"].astype(f32)) * s
    bq = (lb @ inp["wq"].astype(f32) + inp["bq"].astype(f32)) * s
    wk = g[:, None] * inp["wk"].astype(f32)
    wv = g[:, None] * inp["wv"].astype(f32)
    bv = lb @ inp["wv"].astype(f32) + inp["bv"].astype(f32)
    wo = inp["wo"].astype(f32)
    bo = bv @ wo + inp["bo"].astype(f32)
    bf = ml_dtypes.bfloat16
    def wfmt(w):
        return np.ascontiguousarray(w.reshape(2, 128, 256).transpose(1, 0, 2)).astype(bf)
    return {
        "wq": wfmt(wq), "wk": wfmt(wk), "wv": wfmt(wv), "wo": wfmt(wo),
        "bq": np.ascontiguousarray(bq.reshape(2, 128).T).astype(f32),
        "bo": bo.reshape(1, 256).astype(bf),
        "masks": _build_masks_np(),
        "qz": np.zeros((128, FPC * 8192), ml_dtypes.bfloat16),
        "vz": np.ones((128, 9 * NH * 33), ml_dtypes.bfloat16),
    }


# ---------------------------------------------------------------- bass build
def _build_bass():
    from contextlib import ExitStack
    import concourse.tile as tile
    from concourse import bacc, mybir

    dt = mybir.dt
    AF = mybir.ActivationFunctionType
    OP = mybir.AluOpType

    nc = bacc.Bacc("TRN2", target_bir_lowering=False, debug=False,
                   num_devices=N_CORES)

    x_d = nc.dram_tensor("x", [FPC * N, D], dt.float32, kind="ExternalInput").ap()
    wq_d = nc.dram_tensor("wq", [128, 2, 256], dt.bfloat16, kind="ExternalInput").ap()
    wk_d = nc.dram_tensor("wk", [128, 2, 256], dt.bfloat16, kind="ExternalInput").ap()
    wv_d = nc.dram_tensor("wv", [128, 2, 256], dt.bfloat16, kind="ExternalInput").ap()
    wo_d = nc.dram_tensor("wo", [128, 2, 256], dt.bfloat16, kind="ExternalInput").ap()
    bq_d = nc.dram_tensor("bq", [128, 2], dt.float32, kind="ExternalInput").ap()
    bo_d = nc.dram_tensor("bo", [1, 256], dt.bfloat16, kind="ExternalInput").ap()
    mk_d = nc.dram_tensor("masks", [128, 3, 64], dt.bfloat16, kind="ExternalInput").ap()
    qz_d = nc.dram_tensor("qz", [128, FPC * 8192], dt.bfloat16, kind="ExternalInput").ap()
    vz_d = nc.dram_tensor("vz", [128, 9 * NH * 33], dt.bfloat16, kind="ExternalInput").ap()
    y_d = nc.dram_tensor("y", [FPC * N, D], dt.float32, kind="ExternalOutput").ap()

    with tile.TileContext(nc) as tc:
        with ExitStack() as ctx:
            const = ctx.enter_context(tc.tile_pool(name="const", bufs=1))
            frame = ctx.enter_context(tc.tile_pool(name="frame", bufs=1))
            work = ctx.enter_context(tc.tile_pool(name="work", bufs=3))
            att = ctx.enter_context(tc.tile_pool(name="att", bufs=3))
            psc = ctx.enter_context(tc.tile_pool(name="psc", bufs=2, space="PSUM"))
            pav = ctx.enter_context(tc.tile_pool(name="pav", bufs=2, space="PSUM"))
            pgen = ctx.enter_context(tc.tile_pool(name="pgen", bufs=2, space="PSUM"))

            # ---- constants ----
            ident = const.tile([128, 128], dt.bfloat16)
            from concourse.masks import make_identity
            make_identity(nc, ident[:])
            wq_s = const.tile([128, 2, 256], dt.bfloat16)
            wk_s = const.tile([128, 2, 256], dt.bfloat16)
            wv_s = const.tile([128, 2, 256], dt.bfloat16)
            wo_s = const.tile([128, 2, 256], dt.bfloat16)
            bq_s = const.tile([128, 2], dt.float32)
            bo_s = const.tile([1, 256], dt.bfloat16)
            mk_s = const.tile([128, 3, 64], dt.bfloat16)
            ones_s = const.tile([1, 1024], dt.bfloat16)
            nc.vector.memset(ones_s[:], 1.0)
            for cval in (0.0, LN_EPS):
                ct = const.tile([128, 1], dt.float32, tag=f"c{cval}")
                nc.vector.memset(ct[:], cval)
                nc.const_aps.aps[(dt.float32, cval)] = ct[:]

            # ---- persistent per-frame tensors (frame dim f explicit) ----
            x_f = frame.tile([128, FPC, 8, 256], dt.float32)
            xnT = frame.tile([128, FPC, 2, NPAD], dt.bfloat16)
            kTp = frame.tile([128, FPC, 2, NPAD], dt.bfloat16)
            qst = frame.tile([128, FPC, 2, 4, N], dt.bfloat16)
            vau = frame.tile([128, FPC, 9, NH, 33], dt.bfloat16)
            vau64 = frame.tile([128, FPC, 8, NH, 33], dt.bfloat16)
            xoT = frame.tile([128, FPC, 2, N], dt.bfloat16)
            ybuf = frame.tile([128, FPC, 8, 256], dt.float32)
            mv = frame.tile([128, FPC, 8, 2], dt.float32)
            rstd = frame.tile([128, FPC, 8], dt.float32)
            lnv = frame.tile([128, FPC, 8], dt.float32)

            # ---- input loads (per 2-chunk so LN pipelines behind the DMA) ----
            for f in range(FPC):
                for i in range(0, 8, 2):
                    nc.sync.dma_start(
                        x_f[:, f, i:i + 2, :],
                        x_d[f * N + 128 * i:f * N + 128 * (i + 2), :]
                        .rearrange("(i p) d -> p i d", p=128))
            # stripe-zero / ones init straight from DRAM (off the engines)
            for f in range(FPC):
                nc.sync.dma_start(
                    qst[:, f].rearrange("p a b c -> p (a b c)"),
                    qz_d[:, 8192 * f:8192 * (f + 1)])
                nc.sync.dma_start(
                    vau[:, f].rearrange("p a b c -> p (a b c)"), vz_d[:])
                nc.sync.dma_start(
                    vau64[:, f].rearrange("p a b c -> p (a b c)"),
                    vz_d[:, 0:8 * NH * 33])
            for sb, dd in ((wq_s, wq_d), (wk_s, wk_d), (wv_s, wv_d),
                           (wo_s, wo_d), (bq_s, bq_d), (bo_s, bo_d),
                           (mk_s, mk_d)):
                nc.sync.dma_start(sb[:], dd[:])

            # HAM warm-up: dense PE transposes during the otherwise-idle
            # startup window so the clock gate is open at the first matmul
            warm = pgen.tile([128, 256], dt.bfloat16, tag="gen")
            for _ in range(80):
                nc.tensor.transpose(warm[:, 0:128], ident[:], ident[:])

            # pad zeros for xnT / kTp (tiny)
            for f in range(FPC):
                for lo, hi in ((0, 32), (NPAD - 32, NPAD)):
                    nc.vector.memset(xnT[:, f, :, lo:hi], 0.0)
                    nc.vector.memset(kTp[:, f, :, lo:hi], 0.0)

            # ------------- LN + xn + PE transpose into xnT, per frame ------
            for f in range(FPC):
                for i in range(8):
                    st = work.tile([128, 6], dt.float32, tag="bnst")
                    nc.vector.bn_stats(st[:], x_f[:, f, i, :])
                    nc.vector.bn_aggr(mv[:, f, i, :], st[:])
                nc.scalar.activation(lnv[:, f], mv[:, f, :, 1], AF.Ln,
                                     bias=LN_EPS, scale=1.0)
                nc.scalar.activation(rstd[:, f], lnv[:, f], AF.Exp,
                                     bias=0.0, scale=-0.5)
                for u in range(4):
                    ptr = pgen.tile([128, 4, 128], dt.bfloat16, tag="gen")
                    for i2 in range(2):
                        i = 2 * u + i2
                        xn = work.tile([128, 256], dt.bfloat16, tag="xn")
                        nc.vector.tensor_scalar(
                            xn[:], x_f[:, f, i, :], mv[:, f, i, 0:1],
                            rstd[:, f, i:i + 1], OP.subtract, OP.mult)
                        for kc in range(2):
                            nc.tensor.transpose(
                                ptr[:, 2 * i2 + kc, :],
                                xn[:, 128 * kc:128 * (kc + 1)], ident[:])
                    nc.scalar.copy(
                        xnT[:, f, :, 32 + 256 * u:32 + 256 * (u + 1)]
                        .rearrange("p mc (i2 c) -> p i2 mc c", i2=2),
                        ptr[:].rearrange("p (i2 kc) c -> p i2 kc c", i2=2))
                    # PE keep-alive: dep-free weight loads fill the LN-chain
                    # wait so the HAM clock gate stays open
                    for _ in range(4):
                        nc.tensor.ldweights(ident[:])

            # ---------------- q & k & v projections ----------------
            for f in range(FPC):
                for mc in range(2):
                    qnat = work.tile([128, 1024], dt.bfloat16, tag="qnat")
                    for nh in range(2):
                        ns = slice(512 * nh, 512 * (nh + 1))
                        pq = pgen.tile([128, 512], dt.float32, tag="gen")
                        for kc in range(2):
                            nc.tensor.matmul(
                                pq[:], wq_s[:, kc, 128 * mc:128 * (mc + 1)],
                                xnT[:, f, kc, 32 + 512 * nh:32 + 512 * (nh + 1)],
                                start=(kc == 0), stop=(kc == 1))
                        nc.scalar.activation(qnat[:, ns], pq[:], AF.Identity,
                                             bias=bq_s[:, mc:mc + 1], scale=1.0)
                    for g in range(4):
                        nc.sync.dma_start(
                            qst[32 * g:32 * (g + 1), f, mc, g, :],
                            qnat[32 * g:32 * (g + 1), :])
                for mc in range(2):
                    for nh in range(2):
                        pk = pgen.tile([128, 512], dt.float32, tag="gen")
                        for kc in range(2):
                            nc.tensor.matmul(
                                pk[:], wk_s[:, kc, 128 * mc:128 * (mc + 1)],
                                xnT[:, f, kc, 32 + 512 * nh:32 + 512 * (nh + 1)],
                                start=(kc == 0), stop=(kc == 1))
                        nc.vector.tensor_copy(
                            kTp[:, f, mc, 32 + 512 * nh:32 + 512 * (nh + 1)],
                            pk[:])
                for c in range(9):
                    np_ = 128 if c < 8 else 64
                    pvv = pgen.tile([128, 256], dt.float32, tag="gen")
                    for kc in range(2):
                        nc.tensor.matmul(
                            pvv[0:np_, :], xnT[:, f, kc, 128 * c:128 * c + np_],
                            wv_s[:, kc, :], start=(kc == 0), stop=(kc == 1))
                    nc.scalar.copy(
                        vau[0:np_, f, c, :, 0:32],
                        pvv[0:np_, :].rearrange("p (h c) -> p h c", h=NH))
                nc.sync.dma_start(vau64[0:64, f], vau[64:128, f, 0:8])
                nc.sync.dma_start(vau64[64:128, f, 0:8], vau[0:64, f, 1:9])

            # -------- attention, frame-interleaved, out-proj inline --------
            ptro = [None, None]
            for p in range(8):
                for f in range(FPC):
                    pav_t = pav.tile([128, NH, 33], dt.float32, tag="pav")
                    pst = psc.tile([128, 2, 2, 4, 64], dt.float32, tag="sc")
                    for si in range(2):
                        s = 2 * p + si
                        for Q in range(2):
                            nc.tensor.matmul(
                                pst[:, si, Q, :, :],
                                kTp[:, f, Q, 64 * s:64 * s + 128],
                                qst[:, f, Q, :, 64 * s:64 * s + 64],
                                start=True, stop=True)
                    # paired exp + mask over [128, 1024]
                    ae = att.tile([128, NH, 2, 64], dt.bfloat16, tag="ae",
                                  bufs=3)
                    nc.scalar.activation(
                        ae[:].rearrange("p h s j -> p s h j"),
                        pst[:].rearrange("p s q g j -> p s (q g) j"),
                        AF.Exp, bias=0.0, scale=1.0)
                    am = att.tile([128, NH, 2, 64], dt.bfloat16, tag="am",
                                  bufs=6)
                    if p == 0:
                        mask_ap = (mk_s[:, 0:2, :].unsqueeze(1)
                                   .to_broadcast((128, NH, 2, 64)))
                    elif p == 7:
                        mask_ap = (mk_s[:, 1:3, :].unsqueeze(1)
                                   .to_broadcast((128, NH, 2, 64)))
                    else:
                        mask_ap = (mk_s[:, 1:2, :].unsqueeze(2)
                                   .to_broadcast((128, NH, 2, 64)))
                    meng = nc.vector if f == 0 else nc.gpsimd
                    meng.tensor_tensor(am[:], ae[:], mask_ap, OP.mult)
                    for si in range(2):
                        s = 2 * p + si
                        vsrc = (vau[:, f, s // 2] if si == 0
                                else vau64[:, f, (s - 1) // 2])
                        for h in range(NH):
                            nc.tensor.matmul(
                                pav_t[64 * si:64 * (si + 1), h, :],
                                am[:, h, si, :], vsrc[:, h, :],
                                start=True, stop=True)
                    rc = att.tile([128, NH], dt.float32, tag="rc")
                    nc.vector.reciprocal(rc[:], pav_t[:, :, 32])
                    onv = att.tile([128, NH, 32], dt.bfloat16, tag="onv")
                    nc.vector.tensor_tensor(
                        onv[:], pav_t[:, :, 0:32],
                        rc[:].unsqueeze(2).to_broadcast((128, NH, 32)),
                        OP.mult)
                    onf = onv[:].rearrange("p h c -> p (h c)")
                    if p % 2 == 0:
                        ptro[f] = pgen.tile([128, 4, 128], dt.bfloat16,
                                            tag="gen", name=f"ptro{f}")
                    for kc in range(2):
                        nc.tensor.transpose(
                            ptro[f][:, 2 * (p % 2) + kc, :],
                            onf[:, 128 * kc:128 * (kc + 1)], ident[:])
                    # PE keep-alive between pair chains (HAM stays warm)
                    for _ in range(3):
                        nc.tensor.ldweights(ident[:])
                    if p % 2 == 1:
                        u = p // 2
                        nc.scalar.copy(
                            xoT[:, f, :, 256 * u:256 * (u + 1)]
                            .rearrange("p mc (b c) -> p b mc c", b=2),
                            ptro[f][:].rearrange("p (b kc) c -> p b kc c", b=2))
                        for i in (2 * u, 2 * u + 1):
                            py = pgen.tile([128, 256], dt.float32, tag="gen")
                            for kc in range(2):
                                nc.tensor.matmul(
                                    py[:], xoT[:, f, kc, 128 * i:128 * (i + 1)],
                                    wo_s[:, kc, :], start=(kc == 0), stop=False)
                            nc.tensor.matmul(
                                py[:], ones_s[0:1, 0:128], bo_s[:],
                                start=False, stop=True)
                            nc.vector.tensor_tensor(
                                ybuf[:, f, i, :], py[:], x_f[:, f, i, :],
                                OP.add)
            # batched stores (2 x 4-chunk per frame)
            for f in range(FPC):
                for i in (0, 4):
                    nc.sync.dma_start(
                        y_d[f * N + 128 * i:f * N + 128 * (i + 4), :]
                        .rearrange("(i p) d -> p i d", p=128),
                        ybuf[:, f, i:i + 4, :])

    nc.compile()
    return nc


# ---------------------------------------------------------------- entry point
def kernel(**inputs):
    global _COMPILED
    if _COMPILED is None:
        _COMPILED = _build_bass()
    nc = _COMPILED

    from concourse.bass_utils import run_bass_kernel_spmd

    x = np.asarray(inputs["x"], dtype=np.float32)          # [2, 8, 32, 32, 256]
    B, T = x.shape[0], x.shape[1]
    frames = x.reshape(B * T, N, D)
    params = _fold_params({k: np.asarray(v) for k, v in inputs.items()})

    in_maps = []
    for c in range(N_CORES):
        m = {"x": np.ascontiguousarray(
            frames[FPC * c:FPC * (c + 1)].reshape(FPC * N, D))}
        m.update(params)
        in_maps.append(m)

    res = run_bass_kernel_spmd(nc, in_maps, list(range(N_CORES)))
    y = np.concatenate([res.results[c]["y"].reshape(FPC, N, D)
                        for c in range(N_CORES)], axis=0)
    return y.reshape(x.shape).astype(np.float32)


# revision 33
# speedup vs baseline: 1.5375x; 1.0145x over previous
"""Trainium2 Bass kernel for LocalScopeSelfAttention (3x3 window, clamp-padded).

Shapes (hardcoded): x [2, 8, 32, 32, 256] f32, 8 heads x hd=32, LN eps 1e-5.
Sharding: data-parallel over B*T=16 frames -> 2 frames per core on 8 cores.

v6: engine-balance + startup restructure.
  - qst zero-stripes and vau ones come from DRAM constant inputs via DMA
    (no multi-microsecond DVE/GpSimd memsets on the critical path).
  - exp and the clamp-multiplicity mask run per subtile-PAIR ([128,1024]);
    masks on GpSimd (only contends with DVE 2-port ops, which are scarce).
  - xn transposes via the DMA xbar (off the tensor engine); attention output
    transposes stay on PE with ACT evacuation through the shared PSUM ring.
  - The two frames are pair-interleaved through attention; identity-transpose
    warm-up opens the PE clock gate during startup.
"""

import numpy as np
import ml_dtypes

H = W = 32
N = H * W          # 1024 tokens per frame
D = 256
NH, HD = 8, 32
LN_EPS = 1e-5
N_CORES = 8
FPC = 2            # frames per core
NPAD = N + 64      # padded tokens (32 guard each side)

_COMPILED = None


# ---------------------------------------------------------------- host helpers
def _build_masks_np():
    colcount = np.zeros((W, W), np.float32)
    for qc in range(W):
        for dc in (-1, 0, 1):
            colcount[qc, min(max(qc + dc, 0), W - 1)] += 1
    rowcounts = np.zeros((3, 2, 4), np.float32)
    for v, s in ((0, 0), (1, 7), (2, 15)):
        for rq in (0, 1):
            for dh in (-1, 0, 1):
                tgt = min(max(2 * s + rq + dh, 0), H - 1)
                rowcounts[v, rq, tgt - (2 * s - 1)] += 1
    masks = np.zeros((128, 3, 64), np.float32)
    for p in range(128):
        rp, kc = p // 32, p % 32
        for j in range(64):
            rq, qc = j // 32, j % 32
            for v in range(3):
                masks[p, v, j] = rowcounts[v, rq, rp] * colcount[qc, kc]
    return masks.astype(ml_dtypes.bfloat16)


def _fold_params(inp):
    f32 = np.float32
    g = inp["ln_g"].astype(f32)
    lb = inp["ln_b"].astype(f32)
    s = f32(1.0 / np.sqrt(HD))
    wq = (g[:, None] * inp["wq"].astype(f32)) * s
    bq = (lb @ inp["wq"].astype(f32) + inp["bq"].astype(f32)) * s
    wk = g[:, None] * inp["wk"].astype(f32)
    wv = g[:, None] * inp["wv"].astype(f32)
    bv = lb @ inp["wv"].astype(f32) + inp["bv"].astype(f32)
    wo = inp["wo"].astype(f32)
    bo = bv @ wo + inp["bo"].astype(f32)
    bf = ml_dtypes.bfloat16
    def wfmt(w):
        return np.ascontiguousarray(w.reshape(2, 128, 256).transpose(1, 0, 2)).astype(bf)
    return {
        "wq": wfmt(wq), "wk": wfmt(wk), "wv": wfmt(wv), "wo": wfmt(wo),
        "bq": np.ascontiguousarray(bq.reshape(2, 128).T).astype(f32),
        "bo": bo.reshape(1, 256).astype(bf),
        "masks": _build_masks_np(),
        "qz": np.zeros((128, FPC * 8192), ml_dtypes.bfloat16),
        "vz": np.ones((128, 9 * NH * 33), ml_dtypes.bfloat16),
    }


# ---------------------------------------------------------------- bass build
def _build_bass():
    from contextlib import ExitStack
    import concourse.tile as tile
    from concourse import bacc, mybir

    dt = mybir.dt
    AF = mybir.ActivationFunctionType
    OP = mybir.AluOpType

    nc = bacc.Bacc("TRN2", target_bir_lowering=False, debug=False,
                   num_devices=N_CORES)

    x_d = nc.dram_tensor("x", [FPC * N, D], dt.float32, kind="ExternalInput").ap()
    wq_d = nc.dram_tensor("wq", [128, 2, 256], dt.bfloat16, kind="ExternalInput").ap()
    wk_d = nc.dram_tensor("wk", [128, 2, 256], dt.bfloat16, kind="ExternalInput").ap()
    wv_d = nc.dram_tensor("wv", [128, 2, 256], dt.bfloat16, kind="ExternalInput").ap()
    wo_d = nc.dram_tensor("wo", [128, 2, 256], dt.bfloat16, kind="ExternalInput").ap()
    bq_d = nc.dram_tensor("bq", [128, 2], dt.float32, kind="ExternalInput").ap()
    bo_d = nc.dram_tensor("bo", [1, 256], dt.bfloat16, kind="ExternalInput").ap()
    mk_d = nc.dram_tensor("masks", [128, 3, 64], dt.bfloat16, kind="ExternalInput").ap()
    qz_d = nc.dram_tensor("qz", [128, FPC * 8192], dt.bfloat16, kind="ExternalInput").ap()
    vz_d = nc.dram_tensor("vz", [128, 9 * NH * 33], dt.bfloat16, kind="ExternalInput").ap()
    y_d = nc.dram_tensor("y", [FPC * N, D], dt.float32, kind="ExternalOutput").ap()

    with tile.TileContext(nc) as tc:
        with ExitStack() as ctx:
            const = ctx.enter_context(tc.tile_pool(name="const", bufs=1))
            frame = ctx.enter_context(tc.tile_pool(name="frame", bufs=1))
            work = ctx.enter_context(tc.tile_pool(name="work", bufs=3))
            att = ctx.enter_context(tc.tile_pool(name="att", bufs=3))
            psc = ctx.enter_context(tc.tile_pool(name="psc", bufs=2, space="PSUM"))
            pav = ctx.enter_context(tc.tile_pool(name="pav", bufs=2, space="PSUM"))
            pgen = ctx.enter_context(tc.tile_pool(name="pgen", bufs=2, space="PSUM"))

            # ---- constants ----
            ident = const.tile([128, 128], dt.bfloat16)
            from concourse.masks import make_identity
            make_identity(nc, ident[:])
            wq_s = const.tile([128, 2, 256], dt.bfloat16)
            wk_s = const.tile([128, 2, 256], dt.bfloat16)
            wv_s = const.tile([128, 2, 256], dt.bfloat16)
            wo_s = const.tile([128, 2, 256], dt.bfloat16)
            bq_s = const.tile([128, 2], dt.float32)
            bo_s = const.tile([1, 256], dt.bfloat16)
            mk_s = const.tile([128, 3, 64], dt.bfloat16)
            ones_s = const.tile([1, 1024], dt.bfloat16)
            nc.vector.memset(ones_s[:], 1.0)
            for cval in (0.0, LN_EPS):
                ct = const.tile([128, 1], dt.float32, tag=f"c{cval}")
                nc.vector.memset(ct[:], cval)
                nc.const_aps.aps[(dt.float32, cval)] = ct[:]

            # ---- persistent per-frame tensors (frame dim f explicit) ----
            x_f = frame.tile([128, FPC, 8, 256], dt.float32)
            xnT = frame.tile([128, FPC, 2, NPAD], dt.bfloat16)
            kTp = frame.tile([128, FPC, 2, NPAD], dt.bfloat16)
            qst = frame.tile([128, FPC, 2, 4, N], dt.bfloat16)
            vau = frame.tile([128, FPC, 9, NH, 33], dt.bfloat16)
            vau64 = frame.tile([128, FPC, 8, NH, 33], dt.bfloat16)
            xoT = frame.tile([128, FPC, 2, N], dt.bfloat16)
            ybuf = frame.tile([128, FPC, 8, 256], dt.float32)
            mv = frame.tile([128, FPC, 8, 2], dt.float32)
            rstd = frame.tile([128, FPC, 8], dt.float32)
            lnv = frame.tile([128, FPC, 8], dt.float32)

            # ---- input loads (per 2-chunk so LN pipelines behind the DMA) ----
            for f in range(FPC):
                for i in range(0, 8, 2):
                    nc.sync.dma_start(
                        x_f[:, f, i:i + 2, :],
                        x_d[f * N + 128 * i:f * N + 128 * (i + 2), :]
                        .rearrange("(i p) d -> p i d", p=128))
            # stripe-zero / ones init straight from DRAM (off the engines)
            for f in range(FPC):
                nc.sync.dma_start(
                    qst[:, f].rearrange("p a b c -> p (a b c)"),
                    qz_d[:, 8192 * f:8192 * (f + 1)])
                nc.sync.dma_start(
                    vau[:, f].rearrange("p a b c -> p (a b c)"), vz_d[:])
                nc.sync.dma_start(
                    vau64[:, f].rearrange("p a b c -> p (a b c)"),
                    vz_d[:, 0:8 * NH * 33])
            for sb, dd in ((wq_s, wq_d), (wk_s, wk_d), (wv_s, wv_d),
                           (wo_s, wo_d), (bq_s, bq_d), (bo_s, bo_d),
                           (mk_s, mk_d)):
                nc.sync.dma_start(sb[:], dd[:])

            # HAM warm-up: dense PE transposes during the otherwise-idle
            # startup window so the clock gate is open at the first matmul
            warm = pgen.tile([128, 256], dt.bfloat16, tag="gen")
            for _ in range(80):
                nc.tensor.transpose(warm[:, 0:128], ident[:], ident[:])

            # pad zeros for xnT / kTp (tiny)
            for f in range(FPC):
                for lo, hi in ((0, 32), (NPAD - 32, NPAD)):
                    nc.vector.memset(xnT[:, f, :, lo:hi], 0.0)
                    nc.vector.memset(kTp[:, f, :, lo:hi], 0.0)

            # ------------- LN + xn + PE transpose into xnT, per frame ------
            for f in range(FPC):
                for i in range(8):
                    st = work.tile([128, 6], dt.float32, tag="bnst")
                    nc.vector.bn_stats(st[:], x_f[:, f, i, :])
                    nc.vector.bn_aggr(mv[:, f, i, :], st[:])
                nc.scalar.activation(lnv[:, f], mv[:, f, :, 1], AF.Ln,
                                     bias=LN_EPS, scale=1.0)
                nc.scalar.activation(rstd[:, f], lnv[:, f], AF.Exp,
                                     bias=0.0, scale=-0.5)
                for u in range(4):
                    ptr = pgen.tile([128, 4, 128], dt.bfloat16, tag="gen")
                    for i2 in range(2):
                        i = 2 * u + i2
                        xn = work.tile([128, 256], dt.bfloat16, tag="xn")
                        nc.vector.tensor_scalar(
                            xn[:], x_f[:, f, i, :], mv[:, f, i, 0:1],
                            rstd[:, f, i:i + 1], OP.subtract, OP.mult)
                        for kc in range(2):
                            nc.tensor.transpose(
                                ptr[:, 2 * i2 + kc, :],
                                xn[:, 128 * kc:128 * (kc + 1)], ident[:])
                    nc.scalar.copy(
                        xnT[:, f, :, 32 + 256 * u:32 + 256 * (u + 1)]
                        .rearrange("p mc (i2 c) -> p i2 mc c", i2=2),
                        ptr[:].rearrange("p (i2 kc) c -> p i2 kc c", i2=2))
                    # PE keep-alive: dep-free weight loads fill the LN-chain
                    # wait so the HAM clock gate stays open
                    for _ in range(7):
                        nc.tensor.ldweights(ident[:])

            # ---------------- q & k & v projections ----------------
            for f in range(FPC):
                for mc in range(2):
                    qnat = work.tile([128, 1024], dt.bfloat16, tag="qnat")
                    for nh in range(2):
                        ns = slice(512 * nh, 512 * (nh + 1))
                        pq = pgen.tile([128, 512], dt.float32, tag="gen")
                        for kc in range(2):
                            nc.tensor.matmul(
                                pq[:], wq_s[:, kc, 128 * mc:128 * (mc + 1)],
                                xnT[:, f, kc, 32 + 512 * nh:32 + 512 * (nh + 1)],
                                start=(kc == 0), stop=(kc == 1))
                        nc.scalar.activation(qnat[:, ns], pq[:], AF.Identity,
                                             bias=bq_s[:, mc:mc + 1], scale=1.0)
                    for g in range(4):
                        nc.sync.dma_start(
                            qst[32 * g:32 * (g + 1), f, mc, g, :],
                            qnat[32 * g:32 * (g + 1), :])
                for mc in range(2):
                    for nh in range(2):
                        pk = pgen.tile([128, 512], dt.float32, tag="gen")
                        for kc in range(2):
                            nc.tensor.matmul(
                                pk[:], wk_s[:, kc, 128 * mc:128 * (mc + 1)],
                                xnT[:, f, kc, 32 + 512 * nh:32 + 512 * (nh + 1)],
                                start=(kc == 0), stop=(kc == 1))
                        nc.vector.tensor_copy(
                            kTp[:, f, mc, 32 + 512 * nh:32 + 512 * (nh + 1)],
                            pk[:])
                for c in range(9):
                    np_ = 128 if c < 8 else 64
                    pvv = pgen.tile([128, 256], dt.float32, tag="gen")
                    for kc in range(2):
                        nc.tensor.matmul(
                            pvv[0:np_, :], xnT[:, f, kc, 128 * c:128 * c + np_],
                            wv_s[:, kc, :], start=(kc == 0), stop=(kc == 1))
                    nc.scalar.copy(
                        vau[0:np_, f, c, :, 0:32],
                        pvv[0:np_, :].rearrange("p (h c) -> p h c", h=NH))
                nc.sync.dma_start(vau64[0:64, f], vau[64:128, f, 0:8])
                nc.sync.dma_start(vau64[64:128, f, 0:8], vau[0:64, f, 1:9])
                # PE keep-alive across the projection->attention transition
                for _ in range(6):
                    nc.tensor.ldweights(ident[:])

            # -------- attention, frame-interleaved, out-proj inline --------
            ptro = [None, None]
            for p in range(8):
                for f in range(FPC):
                    pav_t = pav.tile([128, NH, 33], dt.float32, tag="pav")
                    pst = psc.tile([128, 2, 2, 4, 64], dt.float32, tag="sc")
                    for si in range(2):
                        s = 2 * p + si
                        for Q in range(2):
                            nc.tensor.matmul(
                                pst[:, si, Q, :, :],
                                kTp[:, f, Q, 64 * s:64 * s + 128],
                                qst[:, f, Q, :, 64 * s:64 * s + 64],
                                start=True, stop=True)
                    # paired exp + mask over [128, 1024]
                    ae = att.tile([128, NH, 2, 64], dt.bfloat16, tag="ae",
                                  bufs=3)
                    nc.scalar.activation(
                        ae[:].rearrange("p h s j -> p s h j"),
                        pst[:].rearrange("p s q g j -> p s (q g) j"),
                        AF.Exp, bias=0.0, scale=1.0)
                    am = att.tile([128, NH, 2, 64], dt.bfloat16, tag="am",
                                  bufs=6)
                    if p == 0:
                        mask_ap = (mk_s[:, 0:2, :].unsqueeze(1)
                                   .to_broadcast((128, NH, 2, 64)))
                    elif p == 7:
                        mask_ap = (mk_s[:, 1:3, :].unsqueeze(1)
                                   .to_broadcast((128, NH, 2, 64)))
                    else:
                        mask_ap = (mk_s[:, 1:2, :].unsqueeze(2)
                                   .to_broadcast((128, NH, 2, 64)))
                    meng = nc.vector if f == 0 else nc.gpsimd
                    meng.tensor_tensor(am[:], ae[:], mask_ap, OP.mult)
                    for si in range(2):
                        s = 2 * p + si
                        vsrc = (vau[:, f, s // 2] if si == 0
                                else vau64[:, f, (s - 1) // 2])
                        for h in range(NH):
                            nc.tensor.matmul(
                                pav_t[64 * si:64 * (si + 1), h, :],
                                am[:, h, si, :], vsrc[:, h, :],
                                start=True, stop=True)
                    rc = att.tile([128, NH], dt.float32, tag="rc")
                    nc.vector.reciprocal(rc[:], pav_t[:, :, 32])
                    onv = att.tile([128, NH, 32], dt.bfloat16, tag="onv")
                    nc.vector.tensor_tensor(
                        onv[:], pav_t[:, :, 0:32],
                        rc[:].unsqueeze(2).to_broadcast((128, NH, 32)),
                        OP.mult)
                    onf = onv[:].rearrange("p h c -> p (h c)")
                    if p % 2 == 0:
                        ptro[f] = pgen.tile([128, 4, 128], dt.bfloat16,
                                            tag="gen", name=f"ptro{f}")
                    for kc in range(2):
                        nc.tensor.transpose(
                            ptro[f][:, 2 * (p % 2) + kc, :],
                            onf[:, 128 * kc:128 * (kc + 1)], ident[:])
                    # PE keep-alive between pair chains (HAM stays warm)
                    for _ in range(4):
                        nc.tensor.ldweights(ident[:])
                    if p % 2 == 1:
                        u = p // 2
                        nc.scalar.copy(
                            xoT[:, f, :, 256 * u:256 * (u + 1)]
                            .rearrange("p mc (b c) -> p b mc c", b=2),
                            ptro[f][:].rearrange("p (b kc) c -> p b kc c", b=2))
                        for i in (2 * u, 2 * u + 1):
                            py = pgen.tile([128, 256], dt.float32, tag="gen")
                            for kc in range(2):
                                nc.tensor.matmul(
                                    py[:], xoT[:, f, kc, 128 * i:128 * (i + 1)],
                                    wo_s[:, kc, :], start=(kc == 0), stop=False)
                            nc.tensor.matmul(
                                py[:], ones_s[0:1, 0:128], bo_s[:],
                                start=False, stop=True)
                            nc.vector.tensor_tensor(
                                ybuf[:, f, i, :], py[:], x_f[:, f, i, :],
                                OP.add)
            # batched stores (2 x 4-chunk per frame)
            for f in range(FPC):
                for i in (0, 4):
                    nc.sync.dma_start(
                        y_d[f * N + 128 * i:f * N + 128 * (i + 4), :]
                        .rearrange("(i p) d -> p i d", p=128),
                        ybuf[:, f, i:i + 4, :])

    nc.compile()
    return nc


# ---------------------------------------------------------------- entry point
def kernel(**inputs):
    global _COMPILED
    if _COMPILED is None:
        _COMPILED = _build_bass()
    nc = _COMPILED

    from concourse.bass_utils import run_bass_kernel_spmd

    x = np.asarray(inputs["x"], dtype=np.float32)          # [2, 8, 32, 32, 256]
    B, T = x.shape[0], x.shape[1]
    frames = x.reshape(B * T, N, D)
    params = _fold_params({k: np.asarray(v) for k, v in inputs.items()})

    in_maps = []
    for c in range(N_CORES):
        m = {"x": np.ascontiguousarray(
            frames[FPC * c:FPC * (c + 1)].reshape(FPC * N, D))}
        m.update(params)
        in_maps.append(m)

    res = run_bass_kernel_spmd(nc, in_maps, list(range(N_CORES)))
    y = np.concatenate([res.results[c]["y"].reshape(FPC, N, D)
                        for c in range(N_CORES)], axis=0)
    return y.reshape(x.shape).astype(np.float32)
